# revision 1
# baseline (speedup 1.0000x reference)
"""Trainium2 Bass kernel for nn_Attention_GLM_Wrapped (S=2048, B=2, D=4096, H=32).

Sharding: 8-way tensor parallel over heads (4 heads/core), both batches on
every core. Per-batch AllToAll redistributes the attention output from
head-sharded to token-sharded form for the output projection; each core
emits the final output rows for its 256-token slice (both batches).

Per-core pipeline (SPMD, identical program, per-core weight shards):
  A) Fused Q/K/V projection in natural [token, e] layout (all three weight
     blocks resident in SBUF, x streamed once in 128-token blocks), bias,
     2D-RoPE on Q/K via free-dim shifted vector ops, PE-transpose of Q/K to
     [d, token] layout, spill to DRAM.
  C) Per (batch, head): logits^T = K^T-chunk @ Q (PSUM), exp on ACT over
     1024-query tiles (no max subtraction; logits are O(10) so exp is safe),
     all-ones-matmul key-sum (replicated over partitions), P@V accumulation,
     normalize with fast reciprocal.  AllToAll for batch b is issued as soon
     as batch b's heads finish, overlapping batch b+1's attention.
  D) Output projection per batch against full attn_out_weight^T; bias.

Matmuls run in float16 (fp32 PSUM accumulation); 4-byte operands stream at
half PE rate on TRN2, so 2-byte operands double matmul throughput vs
fp32/fp32r, and fp16 carries 2 more mantissa bits than bf16. A constant
-10 offset on the logits keeps exp outputs inside fp16 range (the offset
cancels exactly in the softmax normalization).
"""
import os
import sys

sys.path.insert(0, "/opt/trn_rl_repo")

import numpy as np
import ml_dtypes
from contextlib import ExitStack

import concourse.bass as bass
from concourse import bacc
import concourse.mybir as mybir
import concourse.tile as tile
from concourse.bass_utils import run_bass_kernel_spmd
from concourse.masks import make_identity

F32 = mybir.dt.float32
F32R = mybir.dt.float32r
BF16 = mybir.dt.bfloat16
FP16 = mybir.dt.float16
AF = mybir.ActivationFunctionType

MMD = FP16          # matmul operand dtype
EXPB = -10.0        # constant logit offset before exp; cancels in softmax

S, B, D = 2048, 2, 4096
H = 32
HD = 128            # head dim
T = S * B           # 4096 tokens, t = b*S + s
NC = 8              # cores
HPC = H // NC       # 4 heads per core
EH = HPC * HD       # 512 local e-dims per q/k/v
TPC = T // NC // B  # 256 tokens per core per batch (output shard)
SCALE = float(1.0 / np.sqrt(HD))

_cache = {}


def _np_mmd(a):
    if MMD == BF16:
        return np.asarray(a, np.float32).astype(ml_dtypes.bfloat16)
    if MMD == FP16:
        return np.asarray(a, np.float32).astype(np.float16)
    return np.ascontiguousarray(np.asarray(a, np.float32))


def _rope_tables():
    rot = 64
    inv_freq = 1.0 / (10000.0 ** (np.arange(0, rot, 2, dtype=np.float32) / rot))
    v = np.arange(S, dtype=np.float32)[:, None] * inv_freq[None, :]
    v = np.concatenate([v, v], axis=-1)  # [S, 64]
    return np.cos(v).astype(np.float32), np.sin(v).astype(np.float32)


def build_program():
    nc = bacc.Bacc("TRN2", target_bir_lowering=False, debug=False, num_devices=NC)

    xT = nc.dram_tensor("xT", [D, T], MMD, kind="ExternalInput").ap()
    wqT = nc.dram_tensor("wqT", [D, EH], MMD, kind="ExternalInput").ap()
    wkT = nc.dram_tensor("wkT", [D, EH], MMD, kind="ExternalInput").ap()
    wvT = nc.dram_tensor("wvT", [D, EH], MMD, kind="ExternalInput").ap()
    woT = nc.dram_tensor("woT", [D, D], MMD, kind="ExternalInput").ap()
    bq = nc.dram_tensor("bq", [HD, EH], F32, kind="ExternalInput").ap()
    bk = nc.dram_tensor("bk", [HD, EH], F32, kind="ExternalInput").ap()
    bv = nc.dram_tensor("bv", [HD, EH], F32, kind="ExternalInput").ap()
    bo = nc.dram_tensor("bo", [HD, D], F32, kind="ExternalInput").ap()
    cosN = nc.dram_tensor("cosN", [T, HD], F32, kind="ExternalInput").ap()
    sinN = nc.dram_tensor("sinN", [T, HD], F32, kind="ExternalInput").ap()
    onesc = nc.dram_tensor("onesc", [HD, HD], MMD, kind="ExternalInput").ap()
    out = nc.dram_tensor("out", [B, TPC, D], F32, kind="ExternalOutput").ap()
    DEBUG = bool(int(os.environ.get("K_DEBUG", "0")))
    if DEBUG:
        qdump = nc.dram_tensor("qdump", [EH, T], MMD, kind="ExternalOutput").ap()
        kdump = nc.dram_tensor("kdump", [EH, T], MMD, kind="ExternalOutput").ap()
        vdump = nc.dram_tensor("vdump", [T, EH], MMD, kind="ExternalOutput").ap()
        ccdump = nc.dram_tensor("ccdump", [B, NC, EH, TPC], MMD, kind="ExternalOutput").ap()

    NTB = T // HD   # 32 token blocks of 128
    NDC = D // HD   # 32 contraction chunks

    with tile.TileContext(nc) as tc, ExitStack() as top:
        dram = top.enter_context(tc.tile_pool(name="dram", bufs=1, space="DRAM"))
        cpool = top.enter_context(tc.tile_pool(name="cpool", bufs=1))

        qT_d = dram.tile([EH, T], MMD)
        kT_d = dram.tile([EH, T], MMD)
        v_d = dram.tile([T, EH], MMD)
        cc_in = [dram.tile([NC, EH, TPC], MMD, name=f"cc_in_{b}")
                 for b in range(B)]
        cc_out = [dram.tile([NC, EH, TPC], MMD, name=f"cc_out_{b}")
                  for b in range(B)]

        ident = cpool.tile([HD, HD], MMD)
        make_identity(nc, ident)
        ones_sb = cpool.tile([HD, HD], MMD)
        nc.sync.dma_start(ones_sb[:], onesc[:])
        bq_sb = cpool.tile([HD, EH], F32)
        nc.sync.dma_start(bq_sb[:], bq[:])
        bk_sb = cpool.tile([HD, EH], F32)
        nc.sync.dma_start(bk_sb[:], bk[:])
        bv_sb = cpool.tile([HD, EH], F32)
        nc.sync.dma_start(bv_sb[:], bv[:])
        expb_sb = cpool.tile([HD, 1], F32)
        nc.vector.memset(expb_sb[:], EXPB)

        xT_r = xT.rearrange("(o p) t -> p o t", p=HD)        # [128, 32, T]

        # ------- Phase A: fused Q/K/V projection + RoPE + transpose --------
        with ExitStack() as ctx:
            wres = ctx.enter_context(tc.tile_pool(name="wres", bufs=1))
            xp = ctx.enter_context(tc.tile_pool(name="xp", bufs=3))
            rp = ctx.enter_context(tc.tile_pool(name="rp", bufs=3))
            op = ctx.enter_context(tc.tile_pool(name="op", bufs=6))
            ps = ctx.enter_context(tc.tile_pool(name="psA", bufs=4, space="PSUM"))
            pst = ctx.enter_context(tc.tile_pool(name="psAt", bufs=4, space="PSUM"))

            # chunked preloads so the first matmuls only wait for chunk 0
            wqS = wres.tile([HD, NDC, EH], MMD)
            wkS = wres.tile([HD, NDC, EH], MMD)
            wvS = wres.tile([HD, NDC, EH], MMD)
            for ch in range(8):
                csl = slice(ch * NDC // 8, (ch + 1) * NDC // 8)
                for wS, wsrc in ((wqS, wqT), (wkS, wkT), (wvS, wvT)):
                    r = wsrc.rearrange("(o p) e -> p o e", p=HD)
                    nc.sync.dma_start(wS[:, csl], r[:, csl])

            for tb in range(NTB):
                tsl = slice(tb * HD, (tb + 1) * HD)
                xo = xp.tile([HD, NDC, HD], MMD, tag="xo")
                nc.scalar.dma_start(xo[:], xT_r[:, :, tsl])
                cos_t = xp.tile([HD, HD], F32, tag="cos")
                nc.scalar.dma_start(cos_t[:], cosN[tsl, :])
                sin_t = xp.tile([HD, HD], F32, tag="sin")
                nc.scalar.dma_start(sin_t[:], sinN[tsl, :])

                for name, wS, b_sb in (("q", wqS, bq_sb), ("k", wkS, bk_sb)):
                    outd = qT_d if name == "q" else kT_d
                    pq = ps.tile([HD, EH], F32, tag="pqk", name=f"pqk_{name}_{tb}")
                    for d in range(NDC):
                        nc.tensor.matmul(pq[:], xo[:, d], wS[:, d],
                                         start=(d == 0), stop=(d == NDC - 1))
                    qb = rp.tile([HD, EH], F32, tag="qb", name=f"qb_{name}_{tb}")
                    nc.vector.tensor_tensor(
                        qb[:], pq[:], b_sb[:], mybir.AluOpType.add)
                    # rope: rq = qb*cos + shift(qb)*sin_signed
                    rq = rp.tile([HD, EH], MMD, tag="rq", name=f"rq_{name}_{tb}")
                    qb4 = qb.rearrange("p (h e) -> p h e", h=HPC)
                    rq4 = rq.rearrange("p (h e) -> p h e", h=HPC)
                    cosb = cos_t[:, None, :].to_broadcast([HD, HPC, HD])
                    nc.vector.tensor_tensor(rq4[:], qb4[:], cosb,
                                            mybir.AluOpType.mult)
                    qb8 = qb.rearrange("p (h u e) -> p h u e", h=HPC, u=4)
                    rq8 = rq.rearrange("p (h u e) -> p h u e", h=HPC, u=4)
                    sin8 = sin_t.rearrange("p (u e) -> p u e", u=4)
                    tmp = rp.tile([HD, HPC, 2, 32], F32, tag="tmp",
                                  name=f"tmp_{name}_{tb}")
                    nc.vector.tensor_tensor(
                        tmp[:], qb8[:, :, 1::2, :],
                        sin8[:, None, 0::2, :].to_broadcast([HD, HPC, 2, 32]),
                        mybir.AluOpType.mult)
                    nc.vector.tensor_tensor(
                        rq8[:, :, 0::2, :], rq8[:, :, 0::2, :], tmp[:],
                        mybir.AluOpType.add)
                    nc.vector.tensor_tensor(
                        tmp[:], qb8[:, :, 0::2, :],
                        sin8[:, None, 1::2, :].to_broadcast([HD, HPC, 2, 32]),
                        mybir.AluOpType.mult)
                    nc.vector.tensor_tensor(
                        rq8[:, :, 1::2, :], rq8[:, :, 1::2, :], tmp[:],
                        mybir.AluOpType.add)
                    # transpose each head block to [d, tok] and spill
                    for hl in range(HPC):
                        ptr = pst.tile([HD, HD], MMD, tag="ptr",
                                       name=f"ptr_{name}_{tb}_{hl}")
                        nc.tensor.transpose(ptr[:], rq[:, hl * HD:(hl + 1) * HD],
                                            ident[:])
                        ob = op.tile([HD, HD], MMD, tag="ob",
                                     name=f"ob_{name}_{tb}_{hl}")
                        nc.scalar.copy(ob[:], ptr[:])
                        nc.sync.dma_start(outd[hl * HD:(hl + 1) * HD, tsl], ob[:])

                # V: natural layout, bias only
                pv = ps.tile([HD, EH], F32, tag="pqk", name=f"pv_{tb}")
                for d in range(NDC):
                    nc.tensor.matmul(pv[:], xo[:, d], wvS[:, d],
                                     start=(d == 0), stop=(d == NDC - 1))
                vb = op.tile([HD, EH], MMD, tag="vb", name=f"vb_{tb}")
                nc.vector.tensor_tensor(
                    vb[:], pv[:], bv_sb[:], mybir.AluOpType.add)
                nc.sync.dma_start(v_d[tsl, :], vb[:])

        # ------- Phase C: attention per (batch, head) + per-batch A2A ------
        # Phase D pools are opened alongside C so D's input DMAs can be
        # emitted (on the otherwise-idle GPSIMD queue) right after each
        # AllToAll -- every other engine queue is still clogged with phase C
        # work at that point, and HWDGE issuance is in-order per engine.
        NKC = S // HD    # 16 key chunks
        NQT = S // 512   # 4 query tiles of 512
        NES = D // 512   # 8 output column segments
        with ExitStack() as ctx:
            qk = ctx.enter_context(tc.tile_pool(name="qk", bufs=2))
            pp = ctx.enter_context(tc.tile_pool(name="pp", bufs=6))
            ao = ctx.enter_context(tc.tile_pool(name="ao", bufs=4))
            wvf = ctx.enter_context(tc.tile_pool(name="wvf", bufs=1))
            wop = ctx.enter_context(tc.tile_pool(name="wop", bufs=4))
            oo = ctx.enter_context(tc.tile_pool(name="oo", bufs=4))

            bo_sb = wvf.tile([HD, D], F32)
            nc.sync.dma_start(bo_sb[:], bo[:])
            woT_r = woT.rearrange("(o p) e -> p o e", p=HD)  # [128, 32, D]
            wvfS = {}
            wo_first = {}

            with ExitStack() as cps:
                psl = cps.enter_context(tc.tile_pool(name="psl", bufs=4, space="PSUM"))
                pso = cps.enter_context(tc.tile_pool(name="pso", bufs=2, space="PSUM"))
                pss = cps.enter_context(tc.tile_pool(name="pss", bufs=2, space="PSUM"))

                for b in range(B):
                    ssl = slice(b * S, (b + 1) * S)
                    for hl in range(HPC):
                        esl = slice(hl * HD, (hl + 1) * HD)
                        qh = qk.tile([HD, S], MMD, tag="qh", name=f"qh_{b}_{hl}")
                        nc.sync.dma_start(qh[:], qT_d[esl, ssl])
                        kh = qk.tile([HD, S], MMD, tag="kh", name=f"kh_{b}_{hl}")
                        nc.sync.dma_start(kh[:], kT_d[esl, ssl])
                        vh = qk.tile([HD, NKC, HD], MMD, tag="vh", name=f"vh_{b}_{hl}")
                        nc.sync.dma_start(
                            vh[:], v_d[ssl, esl].rearrange("(o p) e -> p o e", p=HD))

                        for qt in range(NQT):
                            qsl = slice(qt * 512, (qt + 1) * 512)
                            po = pso.tile([HD, 512], F32, tag="po",
                                          name=f"po_{b}_{hl}_{qt}")
                            su = pss.tile([HD, 512], F32, tag="su",
                                          name=f"su_{b}_{hl}_{qt}")
                            # software-pipelined: QK for chunk kc is emitted
                            # before exp/ones/PV of chunk kc-1, so the ACT exp
                            # is never at the head of the dependency chain
                            pls = {}

                            def consume(k):
                                pe = pp.tile([HD, 512], MMD, tag="pe",
                                             name=f"pe_{b}_{hl}_{qt}_{k}")
                                nc.scalar.activation(pe[:], pls.pop(k)[:], AF.Exp,
                                                     scale=SCALE, bias=expb_sb[:])
                                nc.tensor.matmul(su[:], ones_sb[:], pe[:],
                                                 start=(k == 0),
                                                 stop=(k == NKC - 1))
                                nc.tensor.matmul(po[:], vh[:, k], pe[:],
                                                 start=(k == 0),
                                                 stop=(k == NKC - 1))

                            for kc in range(NKC):
                                pl = psl.tile([HD, 512], F32, tag="pl",
                                              name=f"pl_{b}_{hl}_{qt}_{kc}")
                                nc.tensor.matmul(
                                    pl[:], kh[:, kc * HD:(kc + 1) * HD], qh[:, qsl],
                                    start=True, stop=True)
                                pls[kc] = pl
                                if kc >= 1:
                                    consume(kc - 1)
                            consume(NKC - 1)
                            rec = ao.tile([HD, 512], F32, tag="rec",
                                          name=f"rec_{b}_{hl}_{qt}")
                            nc.vector.reciprocal_approx_fast(rec[:], su[:])
                            osb = ao.tile([HD, 512], MMD, tag="osb",
                                          name=f"osb_{b}_{hl}_{qt}")
                            nc.vector.tensor_tensor(
                                osb[:], po[:], rec[:], mybir.AluOpType.mult)
                            for j2 in range(2):
                                j = qt * 2 + j2
                                nc.sync.dma_start(
                                    cc_in[b][j, esl, :],
                                    osb[:, j2 * TPC:(j2 + 1) * TPC])
                    # batch b attention done: exchange while b+1 computes
                    nc.gpsimd.collective_compute(
                        "AllToAll", mybir.AluOpType.bypass,
                        replica_groups=[list(range(NC))],
                        ins=[cc_in[b][:]], outs=[cc_out[b][:]],
                    )
                    # prefetch phase-D inputs for this batch on the Pool queue
                    t_ = wvf.tile([HD, NDC, TPC], MMD, name=f"wvfS_{b}")
                    for i in range(NC):
                        nc.gpsimd.dma_start(
                            t_[:, i * HPC:(i + 1) * HPC, :],
                            cc_out[b][i].rearrange("(r1 p) c -> p r1 c", p=HD))
                    wvfS[b] = t_
                    wlo = wop.tile([HD, NDC // 2, 512], MMD, tag="wo",
                                   name=f"wo_pre_lo_{b}")
                    nc.gpsimd.dma_start(wlo[:], woT_r[:, 0:NDC // 2, 0:512])
                    whi = wop.tile([HD, NDC // 2, 512], MMD, tag="wo",
                                   name=f"wo_pre_hi_{b}")
                    nc.gpsimd.dma_start(whi[:], woT_r[:, NDC // 2:NDC, 0:512])
                    wo_first[b] = (wlo, whi)

            if DEBUG:
                nc.sync.dma_start(qdump[:], qT_d[:])
                nc.sync.dma_start(kdump[:], kT_d[:])
                nc.sync.dma_start(vdump[:], v_d[:])
                for b in range(B):
                    nc.sync.dma_start(ccdump[b], cc_out[b][:])

            # ------- Phase D: output projection per batch ------------------
            with ExitStack() as dps:
                ps = dps.enter_context(tc.tile_pool(name="psD", bufs=4, space="PSUM"))
                for b in range(B):
                    for es in range(NES):
                        esl = slice(es * 512, (es + 1) * 512)
                        if es == 0:
                            wo_lo, wo_hi = wo_first[b]
                        else:
                            wo_lo = wop.tile([HD, NDC // 2, 512], MMD, tag="wo")
                            nc.sync.dma_start(wo_lo[:], woT_r[:, 0:NDC // 2, esl])
                            wo_hi = wop.tile([HD, NDC // 2, 512], MMD, tag="wo")
                            nc.sync.dma_start(wo_hi[:], woT_r[:, NDC // 2:NDC, esl])
                        for tb2 in range(TPC // HD):
                            pd = ps.tile([HD, 512], F32, tag="pd",
                                         name=f"pd_{b}_{es}_{tb2}")
                            for d in range(NDC):
                                wo_t = wo_lo if d < NDC // 2 else wo_hi
                                nc.tensor.matmul(
                                    pd[:],
                                    wvfS[b][:, d, tb2 * HD:(tb2 + 1) * HD],
                                    wo_t[:, d % (NDC // 2)],
                                    start=(d == 0), stop=(d == NDC - 1))
                            ob = oo.tile([HD, 512], F32, tag="obD",
                                         name=f"obD_{b}_{es}_{tb2}")
                            nc.vector.tensor_tensor(
                                ob[:], pd[:], bo_sb[:, esl], mybir.AluOpType.add)
                            nc.sync.dma_start(
                                out[b, tb2 * HD:(tb2 + 1) * HD, esl], ob[:])

    nc.compile()
    return nc


def host_prep(x, position_ids, qkv_weight, qkv_bias, attn_out_weight,
              attn_out_bias):
    pos = np.asarray(position_ids).astype(np.int64)
    x = np.asarray(x, dtype=np.float32)
    Wqkv = np.asarray(qkv_weight, dtype=np.float32)
    bqkv = np.asarray(qkv_bias, dtype=np.float32)
    Wo = np.asarray(attn_out_weight, dtype=np.float32)
    bo = np.asarray(attn_out_bias, dtype=np.float32)

    xT = _np_mmd(x.transpose(2, 1, 0).reshape(D, T))
    woT = _np_mmd(Wo.T)

    cos_t, sin_t = _rope_tables()
    cosN = np.empty((T, HD), np.float32)
    sinN = np.empty((T, HD), np.float32)
    for b in range(B):
        rows = slice(b * S, (b + 1) * S)
        p1 = pos[b, 0, :]
        p2 = pos[b, 1, :]
        cosN[rows, 0:64] = cos_t[p1]
        cosN[rows, 64:128] = cos_t[p2]
        s1 = sin_t[p1].copy()
        s1[:, 0:32] *= -1.0
        s2 = sin_t[p2].copy()
        s2[:, 0:32] *= -1.0
        sinN[rows, 0:64] = s1
        sinN[rows, 64:128] = s2

    ones = _np_mmd(np.ones((HD, HD), np.float32))
    shared = dict(xT=xT, woT=woT, cosN=cosN, sinN=sinN, onesc=ones,
                  bo=np.ascontiguousarray(np.broadcast_to(bo, (HD, D))))

    in_maps = []
    for c in range(NC):
        heads = range(HPC * c, HPC * (c + 1))
        wq = np.concatenate([Wqkv[384 * h: 384 * h + 128] for h in heads])
        wk = np.concatenate([Wqkv[384 * h + 128: 384 * h + 256] for h in heads])
        wv = np.concatenate([Wqkv[384 * h + 256: 384 * h + 384] for h in heads])
        in_maps.append(dict(
            shared,
            wqT=_np_mmd(wq.T), wkT=_np_mmd(wk.T), wvT=_np_mmd(wv.T),
            bq=np.ascontiguousarray(np.broadcast_to(np.concatenate(
                [bqkv[384 * h: 384 * h + 128] for h in heads]), (HD, EH))),
            bk=np.ascontiguousarray(np.broadcast_to(np.concatenate(
                [bqkv[384 * h + 128: 384 * h + 256] for h in heads]), (HD, EH))),
            bv=np.ascontiguousarray(np.broadcast_to(np.concatenate(
                [bqkv[384 * h + 256: 384 * h + 384] for h in heads]), (HD, EH))),
        ))
    return in_maps


def kernel(x, position_ids, qkv_weight, qkv_bias, attn_out_weight,
           attn_out_bias, _trace=False):
    if "nc" not in _cache:
        _cache["nc"] = build_program()
    nc = _cache["nc"]

    in_maps = host_prep(x, position_ids, qkv_weight, qkv_bias,
                        attn_out_weight, attn_out_bias)
    res = run_bass_kernel_spmd(nc, in_maps, core_ids=list(range(NC)),
                               trace=_trace)
    _cache["last_result"] = res

    out = np.empty((S, B, D), np.float32)
    for c in range(NC):
        oc = res.results[c]["out"]  # [B, TPC, D]
        for b in range(B):
            out[TPC * c: TPC * (c + 1), b, :] = oc[b]
    return out



# revision 11
# speedup vs baseline: 1.0622x; 1.0622x over previous
"""Trainium2 Bass kernel for nn_Attention_GLM_Wrapped (S=2048, B=2, D=4096, H=32).

Sharding: 8-way tensor parallel over heads (4 heads/core), both batches on
every core. Per-batch AllToAll redistributes the attention output from
head-sharded to token-sharded form for the output projection; each core
emits the final output rows for its 256-token slice (both batches).

The per-core schedule keeps the PE tensor engine saturated (it is the
bottleneck at the power-throttled 13/16 clock):
  A) Q^T/K^T computed DIRECTLY in [dim, token] layout (stationary = weight
     e-block, moving = x chunk) -- no PE transposes.  RoPE + bias fused into
     DVE scalar_tensor_tensor ops reading PSUM with partition-offset (XOR-32)
     input slices; sign folded into the host-built sin table.  V computed in
     natural [token, e] layout (stationary = x chunk, moving = wv).  All
     spilled fp16 to per-(head,batch) DRAM tiles so phase C prefetch of
     batch 0 can start at phase-A midpoint on the idle gpsimd DMA queue.
  C) Per (batch, head): logits^T = K^T-chunk @ Q (PSUM), exp on ACT,
     softmax denominator via fp16 DVE accumulation of the exp tiles plus a
     SINGLE all-ones matmul per query tile (instead of one per key chunk --
     saves 480 PE matmuls), P@V accumulation, fast-reciprocal normalize.
     AllToAll for batch b issued as soon as batch b's heads finish.
  D) Output projection streamed against attn_out_weight^T with 3-segment
     lookahead on the gpsimd DMA queue; middle output-column segments are
     shared between the two batches so wo is only streamed ~1.5x.

Matmuls run in float16 (fp32 PSUM accumulation): 2-byte operands stream at
double rate vs fp32, and fp16 carries 2 more mantissa bits than bf16 (the
QK' logits need them).  A constant -10 offset on the logits keeps exp
outputs inside fp16 range (the offset cancels in the softmax normalize).
"""
import os
import sys

sys.path.insert(0, "/opt/trn_rl_repo")

import numpy as np
import ml_dtypes
from contextlib import ExitStack

import concourse.bass as bass
from concourse import bacc
import concourse.mybir as mybir
import concourse.tile as tile
from concourse.bass_utils import run_bass_kernel_spmd

F32 = mybir.dt.float32
BF16 = mybir.dt.bfloat16
FP16 = mybir.dt.float16
AF = mybir.ActivationFunctionType
ALU = mybir.AluOpType

MMD = FP16          # matmul operand dtype
EXPB = -10.0        # constant logit offset before exp; cancels in softmax

S, B, D = 2048, 2, 4096
H = 32
HD = 128            # head dim
T = S * B           # 4096 tokens, t = b*S + s
NC = 8              # cores
HPC = H // NC       # 4 heads per core
EH = HPC * HD       # 512 local e-dims per q/k/v
TPC = T // NC // B  # 256 tokens per core per batch (output shard)
SCALE = float(1.0 / np.sqrt(HD))

NDC = D // HD       # 32 contraction chunks of 128
NTB = T // 512      # 8 token blocks of 512 (phase A)
NKC = S // HD       # 16 key chunks (phase C)
NQT = S // 512      # 4 query tiles of 512
NES = D // 512      # 8 output column segments (phase D)

_cache = {}


def _np_mmd(a):
    if MMD == BF16:
        return np.asarray(a, np.float32).astype(ml_dtypes.bfloat16)
    if MMD == FP16:
        return np.asarray(a, np.float32).astype(np.float16)
    return np.ascontiguousarray(np.asarray(a, np.float32))


def _rope_tables():
    rot = 64
    inv_freq = 1.0 / (10000.0 ** (np.arange(0, rot, 2, dtype=np.float32) / rot))
    v = np.arange(S, dtype=np.float32)[:, None] * inv_freq[None, :]
    v = np.concatenate([v, v], axis=-1)  # [S, 64]
    return np.cos(v).astype(np.float32), np.sin(v).astype(np.float32)


def build_program():
    nc = bacc.Bacc("TRN2", target_bir_lowering=False, debug=False, num_devices=NC)

    xT = nc.dram_tensor("xT", [D, T], MMD, kind="ExternalInput").ap()
    wqT = nc.dram_tensor("wqT", [D, EH], MMD, kind="ExternalInput").ap()
    wkT = nc.dram_tensor("wkT", [D, EH], MMD, kind="ExternalInput").ap()
    wvT = nc.dram_tensor("wvT", [D, EH], MMD, kind="ExternalInput").ap()
    woT = nc.dram_tensor("woT", [D, D], MMD, kind="ExternalInput").ap()
    bqc = nc.dram_tensor("bqc", [HD, HPC], F32, kind="ExternalInput").ap()
    bkc = nc.dram_tensor("bkc", [HD, HPC], F32, kind="ExternalInput").ap()
    bqs = nc.dram_tensor("bqs", [HD, HPC], F32, kind="ExternalInput").ap()
    bks = nc.dram_tensor("bks", [HD, HPC], F32, kind="ExternalInput").ap()
    bv = nc.dram_tensor("bv", [HD, EH], F32, kind="ExternalInput").ap()
    bo = nc.dram_tensor("bo", [HD, D], F32, kind="ExternalInput").ap()
    cosT = nc.dram_tensor("cosT", [HD, T], F32, kind="ExternalInput").ap()
    sinT = nc.dram_tensor("sinT", [HD, T], F32, kind="ExternalInput").ap()
    onesc = nc.dram_tensor("onesc", [HD, HD], MMD, kind="ExternalInput").ap()
    out = nc.dram_tensor("out", [B, TPC, D], F32, kind="ExternalOutput").ap()
    DEBUG = bool(int(os.environ.get("K_DEBUG", "0")))
    if DEBUG:
        qdump = nc.dram_tensor("qdump", [HPC, B, HD, S], MMD,
                               kind="ExternalOutput").ap()
        kdump = nc.dram_tensor("kdump", [HPC, B, HD, S], MMD,
                               kind="ExternalOutput").ap()
        vdump = nc.dram_tensor("vdump", [B, S, EH], MMD,
                               kind="ExternalOutput").ap()

    with tile.TileContext(nc) as tc, ExitStack() as top:
        dram = top.enter_context(tc.tile_pool(name="dram", bufs=1, space="DRAM"))
        cpool = top.enter_context(tc.tile_pool(name="cpool", bufs=1))
        # top-level so its SBUF space never overlaps phase-A pools: the C
        # prefetch loads run DURING phase A (released-zone overlap deps
        # would otherwise serialize them behind the phase-A pool release)
        qk = top.enter_context(tc.tile_pool(name="qk", bufs=3))

        qT_hb = [[dram.tile([HD, S], MMD, name=f"qT_{h}_{b}") for b in range(B)]
                 for h in range(HPC)]
        kT_hb = [[dram.tile([HD, S], MMD, name=f"kT_{h}_{b}") for b in range(B)]
                 for h in range(HPC)]
        v_db = [dram.tile([S, EH], MMD, name=f"v_{b}") for b in range(B)]
        cc_in = [dram.tile([NC, EH, TPC], MMD, name=f"cc_in_{b}")
                 for b in range(B)]
        cc_out = [dram.tile([NC, EH, TPC], MMD, name=f"cc_out_{b}")
                  for b in range(B)]

        ones_sb = cpool.tile([HD, HD], MMD)
        nc.sync.dma_start(ones_sb[:], onesc[:])
        bqc_sb = cpool.tile([HD, HPC], F32)
        nc.sync.dma_start(bqc_sb[:], bqc[:])
        bkc_sb = cpool.tile([HD, HPC], F32)
        nc.sync.dma_start(bkc_sb[:], bkc[:])
        bqs_sb = cpool.tile([HD, HPC], F32)
        nc.sync.dma_start(bqs_sb[:], bqs[:])
        bks_sb = cpool.tile([HD, HPC], F32)
        nc.sync.dma_start(bks_sb[:], bks[:])
        bv_sb = cpool.tile([HD, EH], F32)
        nc.sync.dma_start(bv_sb[:], bv[:])
        expb_sb = cpool.tile([HD, 1], F32)
        nc.vector.memset(expb_sb[:], EXPB)

        xT_r = xT.rearrange("(o p) t -> p o t", p=HD)        # [128, 32, T]
        woT_r = woT.rearrange("(o p) e -> p o e", p=HD)      # [128, 32, D]

        # ------- Phase A: Q^T/K^T direct + fused RoPE; V natural ----------
        with ExitStack() as ctx:
            wres = ctx.enter_context(tc.tile_pool(name="wres", bufs=1))
            xp = ctx.enter_context(tc.tile_pool(name="xp", bufs=5))
            tp2 = ctx.enter_context(tc.tile_pool(name="tp2", bufs=2))
            rp = ctx.enter_context(tc.tile_pool(name="rp", bufs=2))
            op = ctx.enter_context(tc.tile_pool(name="op", bufs=3))
            ps = ctx.enter_context(tc.tile_pool(name="psA", bufs=4, space="PSUM"))

            # weights resident; chunked per e-block so the first matmuls only
            # wait for their own block (loads on the idle gpsimd DMA queue)
            wqS = wres.tile([HD, NDC, EH], MMD)
            wkS = wres.tile([HD, NDC, EH], MMD)
            wvS = wres.tile([HD, NDC, EH], MMD)
            wq_r = wqT.rearrange("(o p) e -> p o e", p=HD)
            wk_r = wkT.rearrange("(o p) e -> p o e", p=HD)
            wv_r = wvT.rearrange("(o p) e -> p o e", p=HD)
            for eb in range(HPC):
                esl = slice(eb * HD, (eb + 1) * HD)
                nc.gpsimd.dma_start(wqS[:, :, esl], wq_r[:, :, esl])
                nc.gpsimd.dma_start(wkS[:, :, esl], wk_r[:, :, esl])
            for eb in range(HPC):
                esl = slice(eb * HD, (eb + 1) * HD)
                nc.gpsimd.dma_start(wvS[:, :, esl], wv_r[:, :, esl])

            for tb in range(NTB):
                tsl = slice(tb * 512, (tb + 1) * 512)
                bt = tb // (NTB // B)        # batch of this token block
                csl = slice((tb % (NTB // B)) * 512, (tb % (NTB // B)) * 512 + 512)
                # x window in 4 quarter-tiles (8 d-chunks each) to cut SBUF
                # while keeping one-tile-ahead DMA prefetch
                xq = []
                for qtr in range(4):
                    t_ = xp.tile([HD, NDC // 4, 512], MMD, tag="xo",
                                 name=f"xo_{tb}_{qtr}")
                    nc.scalar.dma_start(
                        t_[:], xT_r[:, qtr * (NDC // 4):(qtr + 1) * (NDC // 4),
                                    tsl])
                    xq.append(t_)
                cs = tp2.tile([HD, 512], F32, tag="cos", name=f"cs_{tb}")
                nc.scalar.dma_start(cs[:], cosT[:, tsl])
                sn = tp2.tile([HD, 512], F32, tag="sin", name=f"sn_{tb}")
                nc.scalar.dma_start(sn[:], sinT[:, tsl])

                for name, wS, b_sb, bs_sb, outd in (
                        ("q", wqS, bqc_sb, bqs_sb, qT_hb),
                        ("k", wkS, bkc_sb, bks_sb, kT_hb)):
                    for hl in range(HPC):
                        ebl = slice(hl * HD, (hl + 1) * HD)
                        pq = ps.tile([HD, 512], F32, tag="pq",
                                     name=f"pq_{name}_{tb}_{hl}")
                        for d in range(NDC):
                            nc.tensor.matmul(pq[:], wS[:, d, ebl],
                                             xq[d // 8][:, d % 8],
                                             start=(d == 0), stop=(d == NDC - 1))
                        # rope: rq = (pq+b)*cos + shifted(pq+b)*sin_signed
                        bcol = b_sb[:, hl:hl + 1]
                        rqc = rp.tile([HD, 512], F32, tag="rqc",
                                      name=f"rqc_{name}_{tb}_{hl}")
                        nc.vector.scalar_tensor_tensor(
                            rqc[:], pq[:], bcol, cs[:], ALU.add, ALU.mult)
                        shp = rp.tile([HD, 512], F32, tag="shp",
                                      name=f"shp_{name}_{tb}_{hl}")
                        for blk in range(4):
                            src = blk ^ 1
                            dsl = slice(blk * 32, (blk + 1) * 32)
                            ssl2 = slice(src * 32, (src + 1) * 32)
                            nc.vector.scalar_tensor_tensor(
                                shp[dsl], pq[ssl2], bs_sb[dsl, hl:hl + 1],
                                sn[dsl], ALU.add, ALU.mult)
                        rq = op.tile([HD, 512], MMD, tag="rq",
                                     name=f"rq_{name}_{tb}_{hl}")
                        nc.vector.tensor_tensor(rq[:], rqc[:], shp[:], ALU.add)
                        nc.sync.dma_start(outd[hl][bt][:, csl], rq[:])

                # V: natural layout (stationary = x chunk, moving = wv)
                for st in range(4):
                    stsl = slice(st * HD, (st + 1) * HD)
                    pv = ps.tile([HD, EH], F32, tag="pq", name=f"pv_{tb}_{st}")
                    for d in range(NDC):
                        nc.tensor.matmul(pv[:], xq[d // 8][:, d % 8, stsl],
                                         wvS[:, d],
                                         start=(d == 0), stop=(d == NDC - 1))
                    vb = op.tile([HD, EH], MMD, tag="vb", name=f"vb_{tb}_{st}")
                    nc.vector.tensor_tensor(vb[:], pv[:], bv_sb[:], ALU.add)
                    nc.sync.dma_start(
                        v_db[bt][(tb % (NTB // B)) * 512 + st * HD:
                                 (tb % (NTB // B)) * 512 + (st + 1) * HD, :],
                        vb[:])

        # ------- Phase C + D ----------------------------------------------
        with ExitStack() as ctx:
            pp = ctx.enter_context(tc.tile_pool(name="pp", bufs=4))
            acp = ctx.enter_context(tc.tile_pool(name="acp", bufs=2))
            ao = ctx.enter_context(tc.tile_pool(name="ao", bufs=4))
            wvf = ctx.enter_context(tc.tile_pool(name="wvf", bufs=1))
            wop = ctx.enter_context(tc.tile_pool(name="wop", bufs=3))
            oo = ctx.enter_context(tc.tile_pool(name="oo", bufs=2))

            bo_sb = wvf.tile([HD, D], F32)
            nc.sync.dma_start(bo_sb[:], bo[:])

            # prefetch ALL q/k/v working sets on the (idle) gpsimd queue.
            # batch-0 tiles are complete at phase-A midpoint, so these loads
            # run entirely under phase A and C starts with zero PE bubble.
            qkv_tiles = {}
            for b in range(B):
                for hl in range(HPC):
                    qh = qk.tile([HD, S], MMD, tag="qh", name=f"qh_{b}_{hl}")
                    nc.gpsimd.dma_start(qh[:], qT_hb[hl][b][:])
                    kh = qk.tile([HD, S], MMD, tag="kh", name=f"kh_{b}_{hl}")
                    nc.gpsimd.dma_start(kh[:], kT_hb[hl][b][:])
                    vh = qk.tile([HD, NKC, HD], MMD, tag="vh", name=f"vh_{b}_{hl}")
                    esl = slice(hl * HD, (hl + 1) * HD)
                    nc.gpsimd.dma_start(
                        vh[:], v_db[b][:, esl].rearrange("(o p) e -> p o e", p=HD))
                    qkv_tiles[(b, hl)] = (qh, kh, vh)

            if DEBUG:
                for hl in range(HPC):
                    for b in range(B):
                        nc.sync.dma_start(qdump[hl, b], qT_hb[hl][b][:])
                        nc.sync.dma_start(kdump[hl, b], kT_hb[hl][b][:])
                for b in range(B):
                    nc.sync.dma_start(vdump[b], v_db[b][:])

            # phase D wo-segment schedule: batch 0 alone for es 0-3 (covers
            # the A2A(b1) window), both batches share es 4-7, batch 1 alone
            # for es 0-3.  wo streamed with 3-segment lookahead on gpsimd.
            seg_order = ([(es, (0,)) for es in range(4)]
                         + [(es, (0, 1)) for es in range(4, 8)]
                         + [(es, (1,)) for es in range(4)])
            wo_tiles = []

            def emit_wo_load(idx):
                es, _bs = seg_order[idx]
                esl = slice(es * 512, (es + 1) * 512)
                lo = wop.tile([HD, NDC // 2, 512], MMD, tag="wol",
                              name=f"wo_lo_{idx}")
                nc.gpsimd.dma_start(lo[:], woT_r[:, 0:NDC // 2, esl])
                hi = wop.tile([HD, NDC // 2, 512], MMD, tag="woh",
                              name=f"wo_hi_{idx}")
                nc.gpsimd.dma_start(hi[:], woT_r[:, NDC // 2:NDC, esl])
                wo_tiles.append((lo, hi))

            wvfS = {}
            with ExitStack() as cps:
                psl = cps.enter_context(tc.tile_pool(name="psl", bufs=4,
                                                     space="PSUM"))
                pso = cps.enter_context(tc.tile_pool(name="pso", bufs=2,
                                                     space="PSUM"))
                pss = cps.enter_context(tc.tile_pool(name="pss", bufs=2,
                                                     space="PSUM"))

                for b in range(B):
                    for hl in range(HPC):
                        esl = slice(hl * HD, (hl + 1) * HD)
                        qh, kh, vh = qkv_tiles[(b, hl)]

                        for qt in range(NQT):
                            qsl = slice(qt * 512, (qt + 1) * 512)
                            po = pso.tile([HD, 512], F32, tag="po",
                                          name=f"po_{b}_{hl}_{qt}")
                            pacc = acp.tile([HD, 512], MMD, tag="pacc",
                                            name=f"pacc_{b}_{hl}_{qt}")
                            # software-pipelined: QK for chunk kc is emitted
                            # before exp/accum/PV of chunk kc-1 so the ACT exp
                            # is never at the head of the dependency chain
                            pls = {}

                            def consume(k):
                                pe = pp.tile([HD, 512], MMD, tag="pe",
                                             name=f"pe_{b}_{hl}_{qt}_{k}")
                                nc.scalar.activation(pe[:], pls.pop(k)[:], AF.Exp,
                                                     scale=SCALE, bias=expb_sb[:])
                                if k == 0:
                                    nc.vector.tensor_copy(pacc[:], pe[:])
                                else:
                                    nc.vector.tensor_tensor(
                                        pacc[:], pacc[:], pe[:], ALU.add)
                                nc.tensor.matmul(po[:], vh[:, k], pe[:],
                                                 start=(k == 0),
                                                 stop=(k == NKC - 1))

                            for kc in range(NKC):
                                pl = psl.tile([HD, 512], F32, tag="pl",
                                              name=f"pl_{b}_{hl}_{qt}_{kc}")
                                nc.tensor.matmul(
                                    pl[:], kh[:, kc * HD:(kc + 1) * HD],
                                    qh[:, qsl], start=True, stop=True)
                                pls[kc] = pl
                                if kc >= 1:
                                    consume(kc - 1)
                            consume(NKC - 1)
                            su = pss.tile([HD, 512], F32, tag="su",
                                          name=f"su_{b}_{hl}_{qt}")
                            nc.tensor.matmul(su[:], ones_sb[:], pacc[:],
                                             start=True, stop=True)
                            rec = ao.tile([HD, 512], F32, tag="rec",
                                          name=f"rec_{b}_{hl}_{qt}")
                            nc.vector.reciprocal_approx_fast(rec[:], su[:])
                            osb = ao.tile([HD, 512], MMD, tag="osb",
                                          name=f"osb_{b}_{hl}_{qt}")
                            nc.vector.tensor_tensor(
                                osb[:], po[:], rec[:], ALU.mult)
                            for j2 in range(2):
                                j = qt * 2 + j2
                                nc.sync.dma_start(
                                    cc_in[b][j, esl, :],
                                    osb[:, j2 * TPC:(j2 + 1) * TPC])
                    # batch b attention done: exchange while later work runs
                    nc.gpsimd.collective_compute(
                        "AllToAll", mybir.AluOpType.bypass,
                        replica_groups=[list(range(NC))],
                        ins=[cc_in[b][:]], outs=[cc_out[b][:]],
                    )
                    # phase-D inputs for this batch on the gpsimd queue
                    t_ = wvf.tile([HD, NDC, TPC], MMD, name=f"wvfS_{b}")
                    for i in range(NC):
                        nc.gpsimd.dma_start(
                            t_[:, i * HPC:(i + 1) * HPC, :],
                            cc_out[b][i].rearrange("(r1 p) c -> p r1 c", p=HD))
                    wvfS[b] = t_
                    if b == 0:
                        for idx in range(3):
                            emit_wo_load(idx)
                    else:
                        for idx in range(3, len(seg_order)):
                            emit_wo_load(idx)

            # ------- Phase D: output projection ---------------------------
            with ExitStack() as dps:
                psd = dps.enter_context(tc.tile_pool(name="psD", bufs=4,
                                                     space="PSUM"))
                for idx, (es, bs) in enumerate(seg_order):
                    esl = slice(es * 512, (es + 1) * 512)
                    wo_lo, wo_hi = wo_tiles[idx]
                    for b in bs:
                        for tb2 in range(TPC // HD):
                            pd = psd.tile([HD, 512], F32, tag="pd",
                                          name=f"pd_{idx}_{b}_{tb2}")
                            for d in range(NDC):
                                wo_t = wo_lo if d < NDC // 2 else wo_hi
                                nc.tensor.matmul(
                                    pd[:],
                                    wvfS[b][:, d, tb2 * HD:(tb2 + 1) * HD],
                                    wo_t[:, d % (NDC // 2)],
                                    start=(d == 0), stop=(d == NDC - 1))
                            ob = oo.tile([HD, 512], F32, tag="obD",
                                         name=f"obD_{idx}_{b}_{tb2}")
                            nc.vector.tensor_tensor(
                                ob[:], pd[:], bo_sb[:, esl], ALU.add)
                            nc.sync.dma_start(
                                out[b, tb2 * HD:(tb2 + 1) * HD, esl], ob[:])

    nc.compile()
    return nc


def host_prep(x, position_ids, qkv_weight, qkv_bias, attn_out_weight,
              attn_out_bias):
    pos = np.asarray(position_ids).astype(np.int64)
    x = np.asarray(x, dtype=np.float32)
    Wqkv = np.asarray(qkv_weight, dtype=np.float32)
    bqkv = np.asarray(qkv_bias, dtype=np.float32)
    Wo = np.asarray(attn_out_weight, dtype=np.float32)
    bo = np.asarray(attn_out_bias, dtype=np.float32)

    xT = _np_mmd(x.transpose(2, 1, 0).reshape(D, T))
    woT = _np_mmd(Wo.T)

    cos_t, sin_t = _rope_tables()
    cosN = np.empty((T, HD), np.float32)
    sinN = np.empty((T, HD), np.float32)
    for b in range(B):
        rows = slice(b * S, (b + 1) * S)
        p1 = pos[b, 0, :]
        p2 = pos[b, 1, :]
        cosN[rows, 0:64] = cos_t[p1]
        cosN[rows, 64:128] = cos_t[p2]
        s1 = sin_t[p1].copy()
        s1[:, 0:32] *= -1.0
        s2 = sin_t[p2].copy()
        s2[:, 0:32] *= -1.0
        sinN[rows, 0:64] = s1
        sinN[rows, 64:128] = s2
    cosT = np.ascontiguousarray(cosN.T)   # [128, T]
    sinT = np.ascontiguousarray(sinN.T)

    ones = _np_mmd(np.ones((HD, HD), np.float32))
    shared = dict(xT=xT, woT=woT, cosT=cosT, sinT=sinT, onesc=ones,
                  bo=np.ascontiguousarray(np.broadcast_to(bo, (HD, D))))

    in_maps = []
    for c in range(NC):
        heads = range(HPC * c, HPC * (c + 1))
        wq = np.concatenate([Wqkv[384 * h: 384 * h + 128] for h in heads])
        wk = np.concatenate([Wqkv[384 * h + 128: 384 * h + 256] for h in heads])
        wv = np.concatenate([Wqkv[384 * h + 256: 384 * h + 384] for h in heads])
        bq = np.concatenate([bqkv[384 * h: 384 * h + 128] for h in heads])
        bk = np.concatenate([bqkv[384 * h + 128: 384 * h + 256] for h in heads])
        bvv = np.concatenate([bqkv[384 * h + 256: 384 * h + 384] for h in heads])
        in_maps.append(dict(
            shared,
            wqT=_np_mmd(wq.T), wkT=_np_mmd(wk.T), wvT=_np_mmd(wv.T),
            bqc=np.ascontiguousarray(bq.reshape(HPC, HD).T),
            bkc=np.ascontiguousarray(bk.reshape(HPC, HD).T),
            bqs=np.ascontiguousarray(bq.reshape(HPC, HD).T[np.arange(HD) ^ 32]),
            bks=np.ascontiguousarray(bk.reshape(HPC, HD).T[np.arange(HD) ^ 32]),
            bv=np.ascontiguousarray(np.broadcast_to(bvv, (HD, EH))),
        ))
    return in_maps


def kernel(x, position_ids, qkv_weight, qkv_bias, attn_out_weight,
           attn_out_bias, _trace=False):
    if "nc" not in _cache:
        _cache["nc"] = build_program()
    nc = _cache["nc"]

    in_maps = host_prep(x, position_ids, qkv_weight, qkv_bias,
                        attn_out_weight, attn_out_bias)
    res = run_bass_kernel_spmd(nc, in_maps, core_ids=list(range(NC)),
                               trace=_trace)
    _cache["last_result"] = res

    out = np.empty((S, B, D), np.float32)
    for c in range(NC):
        oc = res.results[c]["out"]  # [B, TPC, D]
        for b in range(B):
            out[TPC * c: TPC * (c + 1), b, :] = oc[b]
    return out


# revision 14
# speedup vs baseline: 1.0737x; 1.0108x over previous
"""Trainium2 Bass kernel for nn_Attention_GLM_Wrapped (S=2048, B=2, D=4096, H=32).

Sharding: 8-way tensor parallel over heads (4 heads/core), both batches on
every core. Per-batch AllToAll redistributes the attention output from
head-sharded to token-sharded form for the output projection; each core
emits the final output rows for its 256-token slice (both batches).

The per-core schedule keeps the PE tensor engine saturated (it is the
bottleneck at the power-throttled 13/16 clock):
  A) Q^T/K^T computed DIRECTLY in [dim, token] layout (stationary = weight
     e-block, moving = x chunk) -- no PE transposes.  RoPE + bias fused into
     DVE scalar_tensor_tensor ops reading PSUM with partition-offset (XOR-32)
     input slices; sign folded into the host-built sin table.  V computed in
     natural [token, e] layout (stationary = x chunk, moving = wv).  All
     spilled fp16 to per-(head,batch) DRAM tiles so phase C prefetch of
     batch 0 can start at phase-A midpoint on the idle gpsimd DMA queue.
  C) Per (batch, head): logits^T = K^T-chunk @ Q (PSUM), exp on ACT,
     softmax denominator via fp16 DVE accumulation of the exp tiles plus a
     SINGLE all-ones matmul per query tile (instead of one per key chunk --
     saves 480 PE matmuls), P@V accumulation, fast-reciprocal normalize.
     AllToAll for batch b issued as soon as batch b's heads finish.
  D) Output projection streamed against attn_out_weight^T with 3-segment
     lookahead on the gpsimd DMA queue; middle output-column segments are
     shared between the two batches so wo is only streamed ~1.5x.

Matmuls run in float16 (fp32 PSUM accumulation): 2-byte operands stream at
double rate vs fp32, and fp16 carries 2 more mantissa bits than bf16 (the
QK' logits need them).  A constant -10 offset on the logits keeps exp
outputs inside fp16 range (the offset cancels in the softmax normalize).
"""
import os
import sys

sys.path.insert(0, "/opt/trn_rl_repo")

import numpy as np
import ml_dtypes
from contextlib import ExitStack

import concourse.bass as bass
from concourse import bacc
import concourse.mybir as mybir
import concourse.tile as tile
from concourse.bass_utils import run_bass_kernel_spmd

F32 = mybir.dt.float32
BF16 = mybir.dt.bfloat16
FP16 = mybir.dt.float16
AF = mybir.ActivationFunctionType
ALU = mybir.AluOpType

MMD = FP16          # matmul operand dtype
EXPB = -10.0        # constant logit offset before exp; cancels in softmax

S, B, D = 2048, 2, 4096
H = 32
HD = 128            # head dim
T = S * B           # 4096 tokens, t = b*S + s
NC = 8              # cores
HPC = H // NC       # 4 heads per core
EH = HPC * HD       # 512 local e-dims per q/k/v
TPC = T // NC // B  # 256 tokens per core per batch (output shard)
SCALE = float(1.0 / np.sqrt(HD))

NDC = D // HD       # 32 contraction chunks of 128
NTB = T // 512      # 8 token blocks of 512 (phase A)
NKC = S // HD       # 16 key chunks (phase C)
NQT = S // 512      # 4 query tiles of 512
NES = D // 512      # 8 output column segments (phase D)

_cache = {}


def _np_mmd(a):
    if MMD == BF16:
        return np.asarray(a, np.float32).astype(ml_dtypes.bfloat16)
    if MMD == FP16:
        return np.asarray(a, np.float32).astype(np.float16)
    return np.ascontiguousarray(np.asarray(a, np.float32))


def _rope_tables():
    rot = 64
    inv_freq = 1.0 / (10000.0 ** (np.arange(0, rot, 2, dtype=np.float32) / rot))
    v = np.arange(S, dtype=np.float32)[:, None] * inv_freq[None, :]
    v = np.concatenate([v, v], axis=-1)  # [S, 64]
    return np.cos(v).astype(np.float32), np.sin(v).astype(np.float32)


def build_program():
    nc = bacc.Bacc("TRN2", target_bir_lowering=False, debug=False, num_devices=NC)

    xT = nc.dram_tensor("xT", [D, T], MMD, kind="ExternalInput").ap()
    wqT = nc.dram_tensor("wqT", [D, EH], MMD, kind="ExternalInput").ap()
    wkT = nc.dram_tensor("wkT", [D, EH], MMD, kind="ExternalInput").ap()
    wvT = nc.dram_tensor("wvT", [D, EH], MMD, kind="ExternalInput").ap()
    woT = nc.dram_tensor("woT", [D, D], MMD, kind="ExternalInput").ap()
    bqc = nc.dram_tensor("bqc", [HD, HPC], F32, kind="ExternalInput").ap()
    bkc = nc.dram_tensor("bkc", [HD, HPC], F32, kind="ExternalInput").ap()
    bqs = nc.dram_tensor("bqs", [HD, HPC], F32, kind="ExternalInput").ap()
    bks = nc.dram_tensor("bks", [HD, HPC], F32, kind="ExternalInput").ap()
    bv = nc.dram_tensor("bv", [HD, EH], F32, kind="ExternalInput").ap()
    bo = nc.dram_tensor("bo", [HD, D], F32, kind="ExternalInput").ap()
    cosT = nc.dram_tensor("cosT", [HD, T], F32, kind="ExternalInput").ap()
    sinT = nc.dram_tensor("sinT", [HD, T], F32, kind="ExternalInput").ap()
    onesc = nc.dram_tensor("onesc", [HD, HD], MMD, kind="ExternalInput").ap()
    out = nc.dram_tensor("out", [B, TPC, D], F32, kind="ExternalOutput").ap()
    DEBUG = bool(int(os.environ.get("K_DEBUG", "0")))
    if DEBUG:
        qdump = nc.dram_tensor("qdump", [HPC, B, HD, S], MMD,
                               kind="ExternalOutput").ap()
        kdump = nc.dram_tensor("kdump", [HPC, B, HD, S], MMD,
                               kind="ExternalOutput").ap()
        vdump = nc.dram_tensor("vdump", [B, S, EH], MMD,
                               kind="ExternalOutput").ap()

    with tile.TileContext(nc) as tc, ExitStack() as top:
        dram = top.enter_context(tc.tile_pool(name="dram", bufs=1, space="DRAM"))
        cpool = top.enter_context(tc.tile_pool(name="cpool", bufs=1))
        # top-level so its SBUF space never overlaps phase-A pools: the C
        # prefetch loads run DURING phase A (released-zone overlap deps
        # would otherwise serialize them behind the phase-A pool release)
        qk = top.enter_context(tc.tile_pool(name="qk", bufs=4))

        qT_hb = [[dram.tile([HD, S], MMD, name=f"qT_{h}_{b}") for b in range(B)]
                 for h in range(HPC)]
        kT_hb = [[dram.tile([HD, S], MMD, name=f"kT_{h}_{b}") for b in range(B)]
                 for h in range(HPC)]
        v_db = [dram.tile([S, EH], MMD, name=f"v_{b}") for b in range(B)]
        cc_in = [dram.tile([NC, EH, TPC], MMD, name=f"cc_in_{b}")
                 for b in range(B)]
        cc_out = [dram.tile([NC, EH, TPC], MMD, name=f"cc_out_{b}")
                  for b in range(B)]

        ones_sb = cpool.tile([HD, HD], MMD)
        nc.sync.dma_start(ones_sb[:], onesc[:])
        bqc_sb = cpool.tile([HD, HPC], F32)
        nc.sync.dma_start(bqc_sb[:], bqc[:])
        bkc_sb = cpool.tile([HD, HPC], F32)
        nc.sync.dma_start(bkc_sb[:], bkc[:])
        bqs_sb = cpool.tile([HD, HPC], F32)
        nc.sync.dma_start(bqs_sb[:], bqs[:])
        bks_sb = cpool.tile([HD, HPC], F32)
        nc.sync.dma_start(bks_sb[:], bks[:])
        bv_sb = cpool.tile([HD, EH], F32)
        nc.sync.dma_start(bv_sb[:], bv[:])
        expb_sb = cpool.tile([HD, 1], F32)
        nc.vector.memset(expb_sb[:], EXPB)

        xT_r = xT.rearrange("(o p) t -> p o t", p=HD)        # [128, 32, T]
        woT_r = woT.rearrange("(o p) e -> p o e", p=HD)      # [128, 32, D]

        # ------- Phase A: Q^T/K^T direct + fused RoPE; V natural ----------
        with ExitStack() as ctx:
            wres = ctx.enter_context(tc.tile_pool(name="wres", bufs=1))
            xp = ctx.enter_context(tc.tile_pool(name="xp", bufs=5))
            tp2 = ctx.enter_context(tc.tile_pool(name="tp2", bufs=2))
            rp = ctx.enter_context(tc.tile_pool(name="rp", bufs=2))
            op = ctx.enter_context(tc.tile_pool(name="op", bufs=2))
            ps = ctx.enter_context(tc.tile_pool(name="psA", bufs=4, space="PSUM"))

            # weights resident; chunked per e-block so the first matmuls only
            # wait for their own block (loads on the idle gpsimd DMA queue)
            wqS = wres.tile([HD, NDC, EH], MMD)
            wkS = wres.tile([HD, NDC, EH], MMD)
            wvS = wres.tile([HD, NDC, EH], MMD)
            wq_r = wqT.rearrange("(o p) e -> p o e", p=HD)
            wk_r = wkT.rearrange("(o p) e -> p o e", p=HD)
            wv_r = wvT.rearrange("(o p) e -> p o e", p=HD)
            for wS, w_r in ((wqS, wq_r), (wkS, wk_r), (wvS, wv_r)):
                for eb in range(HPC):
                    esl = slice(eb * HD, (eb + 1) * HD)
                    nc.gpsimd.dma_start(wS[:, :, esl], w_r[:, :, esl])

            for tb in range(NTB):
                tsl = slice(tb * 512, (tb + 1) * 512)
                bt = tb // (NTB // B)        # batch of this token block
                csl = slice((tb % (NTB // B)) * 512, (tb % (NTB // B)) * 512 + 512)
                # x window in 4 quarter-tiles (8 d-chunks each) to cut SBUF
                # while keeping one-tile-ahead DMA prefetch
                ldq = nc.sync if tb == 0 else nc.scalar
                xq = []
                for qtr in range(4):
                    t_ = xp.tile([HD, NDC // 4, 512], MMD, tag="xo",
                                 name=f"xo_{tb}_{qtr}")
                    ldq.dma_start(
                        t_[:], xT_r[:, qtr * (NDC // 4):(qtr + 1) * (NDC // 4),
                                    tsl])
                    xq.append(t_)
                cs = tp2.tile([HD, 512], F32, tag="cos", name=f"cs_{tb}")
                ldq.dma_start(cs[:], cosT[:, tsl])
                sn = tp2.tile([HD, 512], F32, tag="sin", name=f"sn_{tb}")
                ldq.dma_start(sn[:], sinT[:, tsl])

                for name, wS, b_sb, bs_sb, outd in (
                        ("q", wqS, bqc_sb, bqs_sb, qT_hb),
                        ("k", wkS, bkc_sb, bks_sb, kT_hb)):
                    for hl in range(HPC):
                        ebl = slice(hl * HD, (hl + 1) * HD)
                        pq = ps.tile([HD, 512], F32, tag="pq",
                                     name=f"pq_{name}_{tb}_{hl}")
                        for d in range(NDC):
                            nc.tensor.matmul(pq[:], wS[:, d, ebl],
                                             xq[d // 8][:, d % 8],
                                             start=(d == 0), stop=(d == NDC - 1))
                        # rope: rq = (pq+b)*cos + shifted(pq+b)*sin_signed
                        bcol = b_sb[:, hl:hl + 1]
                        rqc = rp.tile([HD, 512], F32, tag="rqc",
                                      name=f"rqc_{name}_{tb}_{hl}")
                        nc.vector.scalar_tensor_tensor(
                            rqc[:], pq[:], bcol, cs[:], ALU.add, ALU.mult)
                        shp = rp.tile([HD, 512], F32, tag="shp",
                                      name=f"shp_{name}_{tb}_{hl}")
                        for blk in range(4):
                            src = blk ^ 1
                            dsl = slice(blk * 32, (blk + 1) * 32)
                            ssl2 = slice(src * 32, (src + 1) * 32)
                            nc.vector.scalar_tensor_tensor(
                                shp[dsl], pq[ssl2], bs_sb[dsl, hl:hl + 1],
                                sn[dsl], ALU.add, ALU.mult)
                        rq = op.tile([HD, 512], MMD, tag="rq",
                                     name=f"rq_{name}_{tb}_{hl}")
                        nc.vector.tensor_tensor(rq[:], rqc[:], shp[:], ALU.add)
                        nc.sync.dma_start(outd[hl][bt][:, csl], rq[:])

                # V: natural layout (stationary = x chunk, moving = wv)
                for st in range(4):
                    stsl = slice(st * HD, (st + 1) * HD)
                    pv = ps.tile([HD, EH], F32, tag="pq", name=f"pv_{tb}_{st}")
                    for d in range(NDC):
                        nc.tensor.matmul(pv[:], xq[d // 8][:, d % 8, stsl],
                                         wvS[:, d],
                                         start=(d == 0), stop=(d == NDC - 1))
                    vb = op.tile([HD, EH], MMD, tag="vb", name=f"vb_{tb}_{st}")
                    nc.vector.tensor_tensor(vb[:], pv[:], bv_sb[:], ALU.add)
                    nc.sync.dma_start(
                        v_db[bt][(tb % (NTB // B)) * 512 + st * HD:
                                 (tb % (NTB // B)) * 512 + (st + 1) * HD, :],
                        vb[:])

        # ------- Phase C + D ----------------------------------------------
        with ExitStack() as ctx:
            pp = ctx.enter_context(tc.tile_pool(name="pp", bufs=4))
            acp = ctx.enter_context(tc.tile_pool(name="acp", bufs=2))
            ao = ctx.enter_context(tc.tile_pool(name="ao", bufs=4))
            wvf = ctx.enter_context(tc.tile_pool(name="wvf", bufs=1))
            wopl = ctx.enter_context(tc.tile_pool(name="wopl", bufs=3))
            woph = ctx.enter_context(tc.tile_pool(name="woph", bufs=2))
            oo = ctx.enter_context(tc.tile_pool(name="oo", bufs=2))

            bo_sb = wvf.tile([HD, D], F32)
            nc.sync.dma_start(bo_sb[:], bo[:])

            # prefetch ALL q/k/v working sets on the (idle) gpsimd queue.
            # batch-0 tiles are complete at phase-A midpoint, so these loads
            # run entirely under phase A and C starts with zero PE bubble.
            qkv_tiles = {}
            for b in range(B):
                for hl in range(HPC):
                    qh = qk.tile([HD, S], MMD, tag="qh", name=f"qh_{b}_{hl}")
                    nc.gpsimd.dma_start(qh[:], qT_hb[hl][b][:])
                    kh = qk.tile([HD, S], MMD, tag="kh", name=f"kh_{b}_{hl}")
                    nc.gpsimd.dma_start(kh[:], kT_hb[hl][b][:])
                    vh = qk.tile([HD, NKC, HD], MMD, tag="vh", name=f"vh_{b}_{hl}")
                    esl = slice(hl * HD, (hl + 1) * HD)
                    nc.gpsimd.dma_start(
                        vh[:], v_db[b][:, esl].rearrange("(o p) e -> p o e", p=HD))
                    qkv_tiles[(b, hl)] = (qh, kh, vh)

            if DEBUG:
                for hl in range(HPC):
                    for b in range(B):
                        nc.sync.dma_start(qdump[hl, b], qT_hb[hl][b][:])
                        nc.sync.dma_start(kdump[hl, b], kT_hb[hl][b][:])
                for b in range(B):
                    nc.sync.dma_start(vdump[b], v_db[b][:])

            # phase D wo-segment schedule: batch 0 alone for es 0-3 (covers
            # the A2A(b1) window), both batches share es 4-7, batch 1 alone
            # for es 0-3.  wo streamed with 3-segment lookahead on gpsimd.
            seg_order = ([(es, (0,)) for es in range(4)]
                         + [(es, (0, 1)) for es in range(4, 8)]
                         + [(es, (1,)) for es in range(4)])
            wo_tiles = []

            def emit_wo_load(idx):
                es, _bs = seg_order[idx]
                esl = slice(es * 512, (es + 1) * 512)
                lo = wopl.tile([HD, NDC // 2, 512], MMD, tag="wol",
                               name=f"wo_lo_{idx}")
                nc.gpsimd.dma_start(lo[:], woT_r[:, 0:NDC // 2, esl])
                hi = woph.tile([HD, NDC // 2, 512], MMD, tag="woh",
                               name=f"wo_hi_{idx}")
                nc.gpsimd.dma_start(hi[:], woT_r[:, NDC // 2:NDC, esl])
                wo_tiles.append((lo, hi))

            wvfS = {}
            with ExitStack() as cps:
                psl = cps.enter_context(tc.tile_pool(name="psl", bufs=4,
                                                     space="PSUM"))
                pso = cps.enter_context(tc.tile_pool(name="pso", bufs=2,
                                                     space="PSUM"))
                pss = cps.enter_context(tc.tile_pool(name="pss", bufs=2,
                                                     space="PSUM"))

                def finish_qt(b, hl, qt, po, pacc):
                    # su-matmul + normalize for a COMPLETED query tile; called
                    # a few QK matmuls into the next tile so the PE never
                    # waits on the exp->accumulate DVE chain tail
                    su = pss.tile([HD, 512], F32, tag="su",
                                  name=f"su_{b}_{hl}_{qt}")
                    nc.tensor.matmul(su[:], ones_sb[:], pacc[:],
                                     start=True, stop=True)
                    rec = ao.tile([HD, 512], F32, tag="rec",
                                  name=f"rec_{b}_{hl}_{qt}")
                    nc.vector.reciprocal_approx_fast(rec[:], su[:])
                    osb = ao.tile([HD, 512], MMD, tag="osb",
                                  name=f"osb_{b}_{hl}_{qt}")
                    nc.vector.tensor_tensor(osb[:], po[:], rec[:], ALU.mult)
                    esl2 = slice(hl * HD, (hl + 1) * HD)
                    for j2 in range(2):
                        nc.sync.dma_start(
                            cc_in[b][qt * 2 + j2, esl2, :],
                            osb[:, j2 * TPC:(j2 + 1) * TPC])

                for b in range(B):
                    pending = None
                    for hl in range(HPC):
                        qh, kh, vh = qkv_tiles[(b, hl)]

                        for qt in range(NQT):
                            qsl = slice(qt * 512, (qt + 1) * 512)
                            po = pso.tile([HD, 512], F32, tag="po",
                                          name=f"po_{b}_{hl}_{qt}")
                            pacc = acp.tile([HD, 512], MMD, tag="pacc",
                                            name=f"pacc_{b}_{hl}_{qt}")
                            # depth-2 software pipeline: QK for chunks kc and
                            # kc+1 are emitted before exp/accum/PV of chunk
                            # kc-2, giving the ACT exp (+semaphore hops) two
                            # full matmul pairs of latency slack
                            pls = {}

                            def consume(k):
                                pe = pp.tile([HD, 512], MMD, tag="pe",
                                             name=f"pe_{b}_{hl}_{qt}_{k}")
                                nc.scalar.activation(pe[:], pls.pop(k)[:], AF.Exp,
                                                     scale=SCALE, bias=expb_sb[:])
                                if k == 0:
                                    nc.vector.tensor_copy(pacc[:], pe[:])
                                else:
                                    nc.vector.tensor_tensor(
                                        pacc[:], pacc[:], pe[:], ALU.add)
                                nc.tensor.matmul(po[:], vh[:, k], pe[:],
                                                 start=(k == 0),
                                                 stop=(k == NKC - 1))

                            for kc in range(NKC):
                                pl = psl.tile([HD, 512], F32, tag="pl",
                                              name=f"pl_{b}_{hl}_{qt}_{kc}")
                                nc.tensor.matmul(
                                    pl[:], kh[:, kc * HD:(kc + 1) * HD],
                                    qh[:, qsl], start=True, stop=True)
                                pls[kc] = pl
                                if kc == 2 and pending is not None:
                                    finish_qt(*pending)
                                    pending = None
                                if kc >= 2:
                                    consume(kc - 2)
                            consume(NKC - 2)
                            consume(NKC - 1)
                            pending = (b, hl, qt, po, pacc)
                    # flush before the batch AllToAll
                    finish_qt(*pending)
                    # batch b attention done: exchange while later work runs
                    nc.gpsimd.collective_compute(
                        "AllToAll", mybir.AluOpType.bypass,
                        replica_groups=[list(range(NC))],
                        ins=[cc_in[b][:]], outs=[cc_out[b][:]],
                    )
                    # phase-D inputs for this batch on the gpsimd queue
                    t_ = wvf.tile([HD, NDC, TPC], MMD, name=f"wvfS_{b}")
                    for i in range(NC):
                        nc.gpsimd.dma_start(
                            t_[:, i * HPC:(i + 1) * HPC, :],
                            cc_out[b][i].rearrange("(r1 p) c -> p r1 c", p=HD))
                    wvfS[b] = t_
                    if b == 0:
                        for idx in range(3):
                            emit_wo_load(idx)
                    else:
                        for idx in range(3, len(seg_order)):
                            emit_wo_load(idx)

            # ------- Phase D: output projection ---------------------------
            with ExitStack() as dps:
                psd = dps.enter_context(tc.tile_pool(name="psD", bufs=4,
                                                     space="PSUM"))
                for idx, (es, bs) in enumerate(seg_order):
                    esl = slice(es * 512, (es + 1) * 512)
                    wo_lo, wo_hi = wo_tiles[idx]
                    for b in bs:
                        for tb2 in range(TPC // HD):
                            pd = psd.tile([HD, 512], F32, tag="pd",
                                          name=f"pd_{idx}_{b}_{tb2}")
                            for d in range(NDC):
                                wo_t = wo_lo if d < NDC // 2 else wo_hi
                                nc.tensor.matmul(
                                    pd[:],
                                    wvfS[b][:, d, tb2 * HD:(tb2 + 1) * HD],
                                    wo_t[:, d % (NDC // 2)],
                                    start=(d == 0), stop=(d == NDC - 1))
                            ob = oo.tile([HD, 512], F32, tag="obD",
                                         name=f"obD_{idx}_{b}_{tb2}")
                            nc.vector.tensor_tensor(
                                ob[:], pd[:], bo_sb[:, esl], ALU.add)
                            nc.sync.dma_start(
                                out[b, tb2 * HD:(tb2 + 1) * HD, esl], ob[:])

    nc.compile()
    return nc


def host_prep(x, position_ids, qkv_weight, qkv_bias, attn_out_weight,
              attn_out_bias):
    pos = np.asarray(position_ids).astype(np.int64)
    x = np.asarray(x, dtype=np.float32)
    Wqkv = np.asarray(qkv_weight, dtype=np.float32)
    bqkv = np.asarray(qkv_bias, dtype=np.float32)
    Wo = np.asarray(attn_out_weight, dtype=np.float32)
    bo = np.asarray(attn_out_bias, dtype=np.float32)

    xT = _np_mmd(x.transpose(2, 1, 0).reshape(D, T))
    woT = _np_mmd(Wo.T)

    cos_t, sin_t = _rope_tables()
    cosN = np.empty((T, HD), np.float32)
    sinN = np.empty((T, HD), np.float32)
    for b in range(B):
        rows = slice(b * S, (b + 1) * S)
        p1 = pos[b, 0, :]
        p2 = pos[b, 1, :]
        cosN[rows, 0:64] = cos_t[p1]
        cosN[rows, 64:128] = cos_t[p2]
        s1 = sin_t[p1].copy()
        s1[:, 0:32] *= -1.0
        s2 = sin_t[p2].copy()
        s2[:, 0:32] *= -1.0
        sinN[rows, 0:64] = s1
        sinN[rows, 64:128] = s2
    cosT = np.ascontiguousarray(cosN.T)   # [128, T]
    sinT = np.ascontiguousarray(sinN.T)

    ones = _np_mmd(np.ones((HD, HD), np.float32))
    shared = dict(xT=xT, woT=woT, cosT=cosT, sinT=sinT, onesc=ones,
                  bo=np.ascontiguousarray(np.broadcast_to(bo, (HD, D))))

    in_maps = []
    for c in range(NC):
        heads = range(HPC * c, HPC * (c + 1))
        wq = np.concatenate([Wqkv[384 * h: 384 * h + 128] for h in heads])
        wk = np.concatenate([Wqkv[384 * h + 128: 384 * h + 256] for h in heads])
        wv = np.concatenate([Wqkv[384 * h + 256: 384 * h + 384] for h in heads])
        bq = np.concatenate([bqkv[384 * h: 384 * h + 128] for h in heads])
        bk = np.concatenate([bqkv[384 * h + 128: 384 * h + 256] for h in heads])
        bvv = np.concatenate([bqkv[384 * h + 256: 384 * h + 384] for h in heads])
        in_maps.append(dict(
            shared,
            wqT=_np_mmd(wq.T), wkT=_np_mmd(wk.T), wvT=_np_mmd(wv.T),
            bqc=np.ascontiguousarray(bq.reshape(HPC, HD).T),
            bkc=np.ascontiguousarray(bk.reshape(HPC, HD).T),
            bqs=np.ascontiguousarray(bq.reshape(HPC, HD).T[np.arange(HD) ^ 32]),
            bks=np.ascontiguousarray(bk.reshape(HPC, HD).T[np.arange(HD) ^ 32]),
            bv=np.ascontiguousarray(np.broadcast_to(bvv, (HD, EH))),
        ))
    return in_maps


def kernel(x, position_ids, qkv_weight, qkv_bias, attn_out_weight,
           attn_out_bias, _trace=False):
    if "nc" not in _cache:
        _cache["nc"] = build_program()
    nc = _cache["nc"]

    in_maps = host_prep(x, position_ids, qkv_weight, qkv_bias,
                        attn_out_weight, attn_out_bias)
    res = run_bass_kernel_spmd(nc, in_maps, core_ids=list(range(NC)),
                               trace=_trace)
    _cache["last_result"] = res

    out = np.empty((S, B, D), np.float32)
    for c in range(NC):
        oc = res.results[c]["out"]  # [B, TPC, D]
        for b in range(B):
            out[TPC * c: TPC * (c + 1), b, :] = oc[b]
    return out


# revision 16
# speedup vs baseline: 1.0947x; 1.0196x over previous
"""Trainium2 Bass kernel for nn_Attention_GLM_Wrapped (S=2048, B=2, D=4096, H=32).

Sharding: 8-way tensor parallel over heads (4 heads/core), both batches on
every core. Per-batch AllToAll redistributes the attention output from
head-sharded to token-sharded form for the output projection; each core
emits the final output rows for its 256-token slice (both batches).

The per-core schedule keeps the PE tensor engine saturated (it is the
bottleneck at the power-throttled 13/16 clock):
  A) Q^T/K^T computed DIRECTLY in [dim, token] layout (stationary = weight
     e-block, moving = x chunk) -- no PE transposes.  RoPE + bias fused into
     DVE scalar_tensor_tensor ops reading PSUM with partition-offset (XOR-32)
     input slices; sign folded into the host-built sin table.  V computed in
     natural [token, e] layout (stationary = x chunk, moving = wv).  All
     spilled fp16 to per-(head,batch) DRAM tiles so phase C prefetch of
     batch 0 can start at phase-A midpoint on the idle gpsimd DMA queue.
  C) Per (batch, head): logits^T = K^T-chunk @ Q (PSUM), exp on ACT,
     softmax denominator via fp16 DVE accumulation of the exp tiles plus a
     SINGLE all-ones matmul per query tile (instead of one per key chunk --
     saves 480 PE matmuls), P@V accumulation, fast-reciprocal normalize.
     AllToAll for batch b issued as soon as batch b's heads finish.
  D) Output projection streamed against attn_out_weight^T with 3-segment
     lookahead on the gpsimd DMA queue; middle output-column segments are
     shared between the two batches so wo is only streamed ~1.5x.

Matmuls run in float16 (fp32 PSUM accumulation): 2-byte operands stream at
double rate vs fp32, and fp16 carries 2 more mantissa bits than bf16 (the
QK' logits need them).  A constant -10 offset on the logits keeps exp
outputs inside fp16 range (the offset cancels in the softmax normalize).
"""
import os
import sys

sys.path.insert(0, "/opt/trn_rl_repo")

import numpy as np
import ml_dtypes
from contextlib import ExitStack

import concourse.bass as bass
from concourse import bacc
import concourse.mybir as mybir
import concourse.tile as tile
from concourse.bass_utils import run_bass_kernel_spmd

F32 = mybir.dt.float32
BF16 = mybir.dt.bfloat16
FP16 = mybir.dt.float16
AF = mybir.ActivationFunctionType
ALU = mybir.AluOpType

MMD = FP16          # matmul operand dtype
EXPB = -10.0        # constant logit offset before exp; cancels in softmax

S, B, D = 2048, 2, 4096
H = 32
HD = 128            # head dim
T = S * B           # 4096 tokens, t = b*S + s
NC = 8              # cores
HPC = H // NC       # 4 heads per core
EH = HPC * HD       # 512 local e-dims per q/k/v
TPC = T // NC // B  # 256 tokens per core per batch (output shard)
SCALE = float(1.0 / np.sqrt(HD))

NDC = D // HD       # 32 contraction chunks of 128
NTB = T // 512      # 8 token blocks of 512 (phase A)
NKC = S // HD       # 16 key chunks (phase C)
NQT = S // 512      # 4 query tiles of 512
NES = D // 512      # 8 output column segments (phase D)

_cache = {}


def _np_mmd(a):
    if MMD == BF16:
        return np.asarray(a, np.float32).astype(ml_dtypes.bfloat16)
    if MMD == FP16:
        return np.asarray(a, np.float32).astype(np.float16)
    return np.ascontiguousarray(np.asarray(a, np.float32))


def _rope_tables():
    rot = 64
    inv_freq = 1.0 / (10000.0 ** (np.arange(0, rot, 2, dtype=np.float32) / rot))
    v = np.arange(S, dtype=np.float32)[:, None] * inv_freq[None, :]
    v = np.concatenate([v, v], axis=-1)  # [S, 64]
    return np.cos(v).astype(np.float32), np.sin(v).astype(np.float32)


def build_program():
    nc = bacc.Bacc("TRN2", target_bir_lowering=False, debug=False, num_devices=NC)

    xT = nc.dram_tensor("xT", [D, T], MMD, kind="ExternalInput").ap()
    # weights pre-arranged on the host so every DMA reads multi-KB
    # contiguous runs per partition (256B granules crawl at ~170MB/s and
    # clog the DGE rings with tens of thousands of packets)
    wqP = nc.dram_tensor("wqP", [HD, HPC, NDC, HD], MMD,
                         kind="ExternalInput").ap()
    wkP = nc.dram_tensor("wkP", [HD, HPC, NDC, HD], MMD,
                         kind="ExternalInput").ap()
    wvP = nc.dram_tensor("wvP", [HD, NDC, EH], MMD, kind="ExternalInput").ap()
    woP = nc.dram_tensor("woP", [HD, NES, NDC, 512], MMD,
                         kind="ExternalInput").ap()
    bqc = nc.dram_tensor("bqc", [HD, HPC], F32, kind="ExternalInput").ap()
    bkc = nc.dram_tensor("bkc", [HD, HPC], F32, kind="ExternalInput").ap()
    bqs = nc.dram_tensor("bqs", [HD, HPC], F32, kind="ExternalInput").ap()
    bks = nc.dram_tensor("bks", [HD, HPC], F32, kind="ExternalInput").ap()
    bv = nc.dram_tensor("bv", [HD, EH], F32, kind="ExternalInput").ap()
    bo = nc.dram_tensor("bo", [HD, D], F32, kind="ExternalInput").ap()
    cosT = nc.dram_tensor("cosT", [HD, T], F32, kind="ExternalInput").ap()
    sinT = nc.dram_tensor("sinT", [HD, T], F32, kind="ExternalInput").ap()
    onesc = nc.dram_tensor("onesc", [HD, HD], MMD, kind="ExternalInput").ap()
    out = nc.dram_tensor("out", [B, TPC, D], F32, kind="ExternalOutput").ap()
    DEBUG = bool(int(os.environ.get("K_DEBUG", "0")))
    if DEBUG:
        qdump = nc.dram_tensor("qdump", [HPC, B, HD, S], MMD,
                               kind="ExternalOutput").ap()
        kdump = nc.dram_tensor("kdump", [HPC, B, HD, S], MMD,
                               kind="ExternalOutput").ap()
        vdump = nc.dram_tensor("vdump", [B, S, EH], MMD,
                               kind="ExternalOutput").ap()

    with tile.TileContext(nc) as tc, ExitStack() as top:
        dram = top.enter_context(tc.tile_pool(name="dram", bufs=1, space="DRAM"))
        cpool = top.enter_context(tc.tile_pool(name="cpool", bufs=1))
        # top-level so its SBUF space never overlaps phase-A pools: the C
        # prefetch loads run DURING phase A (released-zone overlap deps
        # would otherwise serialize them behind the phase-A pool release)
        qk = top.enter_context(tc.tile_pool(name="qk", bufs=4))

        qT_hb = [[dram.tile([HD, S], MMD, name=f"qT_{h}_{b}") for b in range(B)]
                 for h in range(HPC)]
        kT_hb = [[dram.tile([HD, S], MMD, name=f"kT_{h}_{b}") for b in range(B)]
                 for h in range(HPC)]
        v_db = [dram.tile([S, EH], MMD, name=f"v_{b}") for b in range(B)]
        cc_in = [dram.tile([NC, EH, TPC], MMD, name=f"cc_in_{b}")
                 for b in range(B)]
        cc_out = [dram.tile([NC, EH, TPC], MMD, name=f"cc_out_{b}")
                  for b in range(B)]

        ones_sb = cpool.tile([HD, HD], MMD)
        nc.sync.dma_start(ones_sb[:], onesc[:])
        bqc_sb = cpool.tile([HD, HPC], F32)
        nc.sync.dma_start(bqc_sb[:], bqc[:])
        bkc_sb = cpool.tile([HD, HPC], F32)
        nc.sync.dma_start(bkc_sb[:], bkc[:])
        bqs_sb = cpool.tile([HD, HPC], F32)
        nc.sync.dma_start(bqs_sb[:], bqs[:])
        bks_sb = cpool.tile([HD, HPC], F32)
        nc.sync.dma_start(bks_sb[:], bks[:])
        bv_sb = cpool.tile([HD, EH], F32)
        nc.sync.dma_start(bv_sb[:], bv[:])
        expb_sb = cpool.tile([HD, 1], F32)
        nc.vector.memset(expb_sb[:], EXPB)

        xT_r = xT.rearrange("(o p) t -> p o t", p=HD)        # [128, 32, T]

        # ------- Phase A: Q^T/K^T direct + fused RoPE; V natural ----------
        with ExitStack() as ctx:
            wres = ctx.enter_context(tc.tile_pool(name="wres", bufs=1))
            xp = ctx.enter_context(tc.tile_pool(name="xp", bufs=5))
            tp2 = ctx.enter_context(tc.tile_pool(name="tp2", bufs=2))
            rp = ctx.enter_context(tc.tile_pool(name="rp", bufs=2))
            op = ctx.enter_context(tc.tile_pool(name="op", bufs=2))
            ps = ctx.enter_context(tc.tile_pool(name="psA", bufs=4, space="PSUM"))

            # weights resident; loaded in consumption order, each DMA an
            # 8KB/partition contiguous run (idle gpsimd DMA queue)
            wqS = wres.tile([HD, HPC, NDC, HD], MMD)
            wkS = wres.tile([HD, HPC, NDC, HD], MMD)
            wvS = wres.tile([HD, NDC, EH], MMD)
            for wS, wP in ((wqS, wqP), (wkS, wkP)):
                for eb in range(HPC):
                    nc.gpsimd.dma_start(wS[:, eb], wP[:, eb])
            for oc in range(8):
                osl = slice(oc * (NDC // 8), (oc + 1) * (NDC // 8))
                nc.gpsimd.dma_start(wvS[:, osl], wvP[:, osl])

            for tb in range(NTB):
                tsl = slice(tb * 512, (tb + 1) * 512)
                bt = tb // (NTB // B)        # batch of this token block
                csl = slice((tb % (NTB // B)) * 512, (tb % (NTB // B)) * 512 + 512)
                # x window in 4 quarter-tiles (8 d-chunks each) to cut SBUF
                # while keeping one-tile-ahead DMA prefetch
                ldq = nc.sync if tb == 0 else nc.scalar
                xq = []
                for qtr in range(4):
                    t_ = xp.tile([HD, NDC // 4, 512], MMD, tag="xo",
                                 name=f"xo_{tb}_{qtr}")
                    ldq.dma_start(
                        t_[:], xT_r[:, qtr * (NDC // 4):(qtr + 1) * (NDC // 4),
                                    tsl])
                    xq.append(t_)
                cs = tp2.tile([HD, 512], F32, tag="cos", name=f"cs_{tb}")
                ldq.dma_start(cs[:], cosT[:, tsl])
                sn = tp2.tile([HD, 512], F32, tag="sin", name=f"sn_{tb}")
                ldq.dma_start(sn[:], sinT[:, tsl])

                for name, wS, b_sb, bs_sb, outd in (
                        ("q", wqS, bqc_sb, bqs_sb, qT_hb),
                        ("k", wkS, bkc_sb, bks_sb, kT_hb)):
                    for hl in range(HPC):
                        pq = ps.tile([HD, 512], F32, tag="pq",
                                     name=f"pq_{name}_{tb}_{hl}")
                        for d in range(NDC):
                            nc.tensor.matmul(pq[:], wS[:, hl, d],
                                             xq[d // 8][:, d % 8],
                                             start=(d == 0), stop=(d == NDC - 1))
                        # rope: rq = (pq+b)*cos + shifted(pq+b)*sin_signed
                        bcol = b_sb[:, hl:hl + 1]
                        rqc = rp.tile([HD, 512], F32, tag="rqc",
                                      name=f"rqc_{name}_{tb}_{hl}")
                        nc.vector.scalar_tensor_tensor(
                            rqc[:], pq[:], bcol, cs[:], ALU.add, ALU.mult)
                        shp = rp.tile([HD, 512], F32, tag="shp",
                                      name=f"shp_{name}_{tb}_{hl}")
                        for blk in range(4):
                            src = blk ^ 1
                            dsl = slice(blk * 32, (blk + 1) * 32)
                            ssl2 = slice(src * 32, (src + 1) * 32)
                            nc.vector.scalar_tensor_tensor(
                                shp[dsl], pq[ssl2], bs_sb[dsl, hl:hl + 1],
                                sn[dsl], ALU.add, ALU.mult)
                        rq = op.tile([HD, 512], MMD, tag="rq",
                                     name=f"rq_{name}_{tb}_{hl}")
                        nc.vector.tensor_tensor(rq[:], rqc[:], shp[:], ALU.add)
                        nc.sync.dma_start(outd[hl][bt][:, csl], rq[:])

                # V: natural layout (stationary = x chunk, moving = wv)
                for st in range(4):
                    stsl = slice(st * HD, (st + 1) * HD)
                    pv = ps.tile([HD, EH], F32, tag="pq", name=f"pv_{tb}_{st}")
                    for d in range(NDC):
                        nc.tensor.matmul(pv[:], xq[d // 8][:, d % 8, stsl],
                                         wvS[:, d],
                                         start=(d == 0), stop=(d == NDC - 1))
                    vb = op.tile([HD, EH], MMD, tag="vb", name=f"vb_{tb}_{st}")
                    nc.vector.tensor_tensor(vb[:], pv[:], bv_sb[:], ALU.add)
                    nc.sync.dma_start(
                        v_db[bt][(tb % (NTB // B)) * 512 + st * HD:
                                 (tb % (NTB // B)) * 512 + (st + 1) * HD, :],
                        vb[:])

        # ------- Phase C + D ----------------------------------------------
        with ExitStack() as ctx:
            pp = ctx.enter_context(tc.tile_pool(name="pp", bufs=4))
            acp = ctx.enter_context(tc.tile_pool(name="acp", bufs=2))
            ao = ctx.enter_context(tc.tile_pool(name="ao", bufs=4))
            wvf = ctx.enter_context(tc.tile_pool(name="wvf", bufs=1))
            wopl = ctx.enter_context(tc.tile_pool(name="wopl", bufs=3))
            woph = ctx.enter_context(tc.tile_pool(name="woph", bufs=2))
            oo = ctx.enter_context(tc.tile_pool(name="oo", bufs=2))

            bo_sb = wvf.tile([HD, D], F32)
            nc.sync.dma_start(bo_sb[:], bo[:])

            # prefetch ALL q/k/v working sets on the (idle) gpsimd queue.
            # batch-0 tiles are complete at phase-A midpoint, so these loads
            # run entirely under phase A and C starts with zero PE bubble.
            qkv_tiles = {}

            def emit_qkv_loads(b):
                for hl in range(HPC):
                    qh = qk.tile([HD, S], MMD, tag="qh", name=f"qh_{b}_{hl}")
                    nc.gpsimd.dma_start(qh[:], qT_hb[hl][b][:])
                    kh = qk.tile([HD, S], MMD, tag="kh", name=f"kh_{b}_{hl}")
                    nc.gpsimd.dma_start(kh[:], kT_hb[hl][b][:])
                    vh = qk.tile([HD, NKC, HD], MMD, tag="vh", name=f"vh_{b}_{hl}")
                    esl = slice(hl * HD, (hl + 1) * HD)
                    nc.gpsimd.dma_start(
                        vh[:], v_db[b][:, esl].rearrange("(o p) e -> p o e", p=HD))
                    qkv_tiles[(b, hl)] = (qh, kh, vh)

            emit_qkv_loads(0)

            if DEBUG:
                for hl in range(HPC):
                    for b in range(B):
                        nc.sync.dma_start(qdump[hl, b], qT_hb[hl][b][:])
                        nc.sync.dma_start(kdump[hl, b], kT_hb[hl][b][:])
                for b in range(B):
                    nc.sync.dma_start(vdump[b], v_db[b][:])

            # phase D wo-segment schedule: batch 0 alone for es 0-3 (covers
            # the A2A(b1) window), both batches share es 4-7, batch 1 alone
            # for es 0-3.  wo streamed with 3/2-segment lookahead on gpsimd;
            # the first three segments are emitted BEFORE the batch-1 qkv
            # prefetches so their DGE-ring slots precede the fine-grained
            # wvfS packets and they land during phase C(b0).
            seg_order = ([(es, (0,)) for es in range(4)]
                         + [(es, (0, 1)) for es in range(4, 8)]
                         + [(es, (1,)) for es in range(4)])
            wo_tiles = []

            def emit_wo_load(idx):
                es, _bs = seg_order[idx]
                lo = wopl.tile([HD, NDC // 2, 512], MMD, tag="wol",
                               name=f"wo_lo_{idx}")
                nc.gpsimd.dma_start(lo[:], woP[:, es, 0:NDC // 2])
                hi = woph.tile([HD, NDC // 2, 512], MMD, tag="woh",
                               name=f"wo_hi_{idx}")
                nc.gpsimd.dma_start(hi[:], woP[:, es, NDC // 2:NDC])
                wo_tiles.append((lo, hi))

            for idx in range(2):
                emit_wo_load(idx)
            emit_qkv_loads(1)

            wvfS = {}
            with ExitStack() as cps:
                psl = cps.enter_context(tc.tile_pool(name="psl", bufs=4,
                                                     space="PSUM"))
                pso = cps.enter_context(tc.tile_pool(name="pso", bufs=2,
                                                     space="PSUM"))
                pss = cps.enter_context(tc.tile_pool(name="pss", bufs=2,
                                                     space="PSUM"))

                def finish_qt(b, hl, qt, po, pacc):
                    # su-matmul + normalize for a COMPLETED query tile; called
                    # a few QK matmuls into the next tile so the PE never
                    # waits on the exp->accumulate DVE chain tail
                    su = pss.tile([HD, 512], F32, tag="su",
                                  name=f"su_{b}_{hl}_{qt}")
                    nc.tensor.matmul(su[:], ones_sb[:], pacc[:],
                                     start=True, stop=True)
                    rec = ao.tile([HD, 512], F32, tag="rec",
                                  name=f"rec_{b}_{hl}_{qt}")
                    nc.vector.reciprocal_approx_fast(rec[:], su[:])
                    osb = ao.tile([HD, 512], MMD, tag="osb",
                                  name=f"osb_{b}_{hl}_{qt}")
                    nc.vector.tensor_tensor(osb[:], po[:], rec[:], ALU.mult)
                    esl2 = slice(hl * HD, (hl + 1) * HD)
                    for j2 in range(2):
                        nc.sync.dma_start(
                            cc_in[b][qt * 2 + j2, esl2, :],
                            osb[:, j2 * TPC:(j2 + 1) * TPC])

                for b in range(B):
                    pending = None
                    for hl in range(HPC):
                        qh, kh, vh = qkv_tiles[(b, hl)]

                        for qt in range(NQT):
                            qsl = slice(qt * 512, (qt + 1) * 512)
                            po = pso.tile([HD, 512], F32, tag="po",
                                          name=f"po_{b}_{hl}_{qt}")
                            pacc = acp.tile([HD, 512], MMD, tag="pacc",
                                            name=f"pacc_{b}_{hl}_{qt}")
                            # depth-2 software pipeline: QK for chunks kc and
                            # kc+1 are emitted before exp/accum/PV of chunk
                            # kc-2, giving the ACT exp (+semaphore hops) two
                            # full matmul pairs of latency slack
                            pls = {}

                            def consume(k):
                                pe = pp.tile([HD, 512], MMD, tag="pe",
                                             name=f"pe_{b}_{hl}_{qt}_{k}")
                                nc.scalar.activation(pe[:], pls.pop(k)[:], AF.Exp,
                                                     scale=SCALE, bias=expb_sb[:])
                                if k == 0:
                                    nc.vector.tensor_copy(pacc[:], pe[:])
                                else:
                                    nc.vector.tensor_tensor(
                                        pacc[:], pacc[:], pe[:], ALU.add)
                                nc.tensor.matmul(po[:], vh[:, k], pe[:],
                                                 start=(k == 0),
                                                 stop=(k == NKC - 1))

                            for kc in range(NKC):
                                pl = psl.tile([HD, 512], F32, tag="pl",
                                              name=f"pl_{b}_{hl}_{qt}_{kc}")
                                nc.tensor.matmul(
                                    pl[:], kh[:, kc * HD:(kc + 1) * HD],
                                    qh[:, qsl], start=True, stop=True)
                                pls[kc] = pl
                                if kc == 2 and pending is not None:
                                    finish_qt(*pending)
                                    pending = None
                                if kc >= 2:
                                    consume(kc - 2)
                            consume(NKC - 2)
                            consume(NKC - 1)
                            pending = (b, hl, qt, po, pacc)
                    # flush before the batch AllToAll
                    finish_qt(*pending)
                    # batch b attention done: exchange while later work runs
                    nc.gpsimd.collective_compute(
                        "AllToAll", mybir.AluOpType.bypass,
                        replica_groups=[list(range(NC))],
                        ins=[cc_in[b][:]], outs=[cc_out[b][:]],
                    )
                    # phase-D inputs for this batch on the gpsimd queue
                    t_ = wvf.tile([HD, NDC, TPC], MMD, name=f"wvfS_{b}")
                    for i in range(NC):
                        nc.gpsimd.dma_start(
                            t_[:, i * HPC:(i + 1) * HPC, :],
                            cc_out[b][i].rearrange("(r1 p) c -> p r1 c", p=HD))
                    wvfS[b] = t_
                    if b == 0:
                        emit_wo_load(2)
                    else:
                        for idx in range(3, len(seg_order)):
                            emit_wo_load(idx)

            # ------- Phase D: output projection ---------------------------
            with ExitStack() as dps:
                psd = dps.enter_context(tc.tile_pool(name="psD", bufs=4,
                                                     space="PSUM"))
                for idx, (es, bs) in enumerate(seg_order):
                    esl = slice(es * 512, (es + 1) * 512)
                    wo_lo, wo_hi = wo_tiles[idx]
                    for b in bs:
                        for tb2 in range(TPC // HD):
                            pd = psd.tile([HD, 512], F32, tag="pd",
                                          name=f"pd_{idx}_{b}_{tb2}")
                            for d in range(NDC):
                                wo_t = wo_lo if d < NDC // 2 else wo_hi
                                nc.tensor.matmul(
                                    pd[:],
                                    wvfS[b][:, d, tb2 * HD:(tb2 + 1) * HD],
                                    wo_t[:, d % (NDC // 2)],
                                    start=(d == 0), stop=(d == NDC - 1))
                            ob = oo.tile([HD, 512], F32, tag="obD",
                                         name=f"obD_{idx}_{b}_{tb2}")
                            nc.vector.tensor_tensor(
                                ob[:], pd[:], bo_sb[:, esl], ALU.add)
                            nc.sync.dma_start(
                                out[b, tb2 * HD:(tb2 + 1) * HD, esl], ob[:])

    nc.compile()
    return nc


def host_prep(x, position_ids, qkv_weight, qkv_bias, attn_out_weight,
              attn_out_bias):
    pos = np.asarray(position_ids).astype(np.int64)
    x = np.asarray(x, dtype=np.float32)
    Wqkv = np.asarray(qkv_weight, dtype=np.float32)
    bqkv = np.asarray(qkv_bias, dtype=np.float32)
    Wo = np.asarray(attn_out_weight, dtype=np.float32)
    bo = np.asarray(attn_out_bias, dtype=np.float32)

    xT = _np_mmd(x.transpose(2, 1, 0).reshape(D, T))
    woP = _np_mmd(Wo.reshape(NES, 512, NDC, HD).transpose(3, 0, 2, 1))

    cos_t, sin_t = _rope_tables()
    cosN = np.empty((T, HD), np.float32)
    sinN = np.empty((T, HD), np.float32)
    for b in range(B):
        rows = slice(b * S, (b + 1) * S)
        p1 = pos[b, 0, :]
        p2 = pos[b, 1, :]
        cosN[rows, 0:64] = cos_t[p1]
        cosN[rows, 64:128] = cos_t[p2]
        s1 = sin_t[p1].copy()
        s1[:, 0:32] *= -1.0
        s2 = sin_t[p2].copy()
        s2[:, 0:32] *= -1.0
        sinN[rows, 0:64] = s1
        sinN[rows, 64:128] = s2
    cosT = np.ascontiguousarray(cosN.T)   # [128, T]
    sinT = np.ascontiguousarray(sinN.T)

    ones = _np_mmd(np.ones((HD, HD), np.float32))
    shared = dict(xT=xT, woP=woP, cosT=cosT, sinT=sinT, onesc=ones,
                  bo=np.ascontiguousarray(np.broadcast_to(bo, (HD, D))))

    in_maps = []
    for c in range(NC):
        heads = range(HPC * c, HPC * (c + 1))
        wq = np.concatenate([Wqkv[384 * h: 384 * h + 128] for h in heads])
        wk = np.concatenate([Wqkv[384 * h + 128: 384 * h + 256] for h in heads])
        wv = np.concatenate([Wqkv[384 * h + 256: 384 * h + 384] for h in heads])
        bq = np.concatenate([bqkv[384 * h: 384 * h + 128] for h in heads])
        bk = np.concatenate([bqkv[384 * h + 128: 384 * h + 256] for h in heads])
        bvv = np.concatenate([bqkv[384 * h + 256: 384 * h + 384] for h in heads])
        in_maps.append(dict(
            shared,
            wqP=_np_mmd(wq.reshape(HPC, HD, NDC, HD).transpose(3, 0, 2, 1)),
            wkP=_np_mmd(wk.reshape(HPC, HD, NDC, HD).transpose(3, 0, 2, 1)),
            wvP=_np_mmd(wv.reshape(EH, NDC, HD).transpose(2, 1, 0)),
            bqc=np.ascontiguousarray(bq.reshape(HPC, HD).T),
            bkc=np.ascontiguousarray(bk.reshape(HPC, HD).T),
            bqs=np.ascontiguousarray(bq.reshape(HPC, HD).T[np.arange(HD) ^ 32]),
            bks=np.ascontiguousarray(bk.reshape(HPC, HD).T[np.arange(HD) ^ 32]),
            bv=np.ascontiguousarray(np.broadcast_to(bvv, (HD, EH))),
        ))
    return in_maps


def kernel(x, position_ids, qkv_weight, qkv_bias, attn_out_weight,
           attn_out_bias, _trace=False):
    if "nc" not in _cache:
        _cache["nc"] = build_program()
    nc = _cache["nc"]

    in_maps = host_prep(x, position_ids, qkv_weight, qkv_bias,
                        attn_out_weight, attn_out_bias)
    res = run_bass_kernel_spmd(nc, in_maps, core_ids=list(range(NC)),
                               trace=_trace)
    _cache["last_result"] = res

    out = np.empty((S, B, D), np.float32)
    for c in range(NC):
        oc = res.results[c]["out"]  # [B, TPC, D]
        for b in range(B):
            out[TPC * c: TPC * (c + 1), b, :] = oc[b]
    return out
